# revision 10
# baseline (speedup 1.0000x reference)
"""Two-layer GAT on 8 TRN2 cores — v4: dma_gather edge phase.

Edge phase redesign vs v3 (indirect_dma_start, 1.4us per 128 rows):
  * Per core, dst nodes in NG groups of 128 (one PSUM row each).  Each
    group's edges are split by src < SPLIT (int16 index limit of
    dma_gather) into A/B runs, sorted by src, padded to 128-token
    columns.  Two token streams (A then B) are fetched with ~1024-token
    dma_gather instructions round-robined over 4 SWDGE queues
    (~3.1 ns/token measured vs ~11 ns/token for indirect DMA).
  * Gathered row = [z | es] bf16 of the edge's src node.  Token t lands
    at partition t%128, free slot t//128.
  * Per column (128 tokens): one-hot oh[t,d] = (dstslot[t]==d) built on
    DVE; ohT via PE transpose (matmul with identity); per-token ed via
    matmul(lhsT=ohT, rhs=ed_group); w = exp(leakyrelu(es+ed)) on
    DVE/ACT; messages m = [w*z | w] on DVE; aggregation via
    matmul(lhsT=oh, rhs=m) accumulated in a per-group PSUM tile.
  * Phase A results are parked in SBUF; phase B accumulates its own
    PSUM tile; epilogue adds both, divides by the summed weights,
    applies ELU (layer 1) and writes 128 output rows sequentially (no
    indirect scatter).
  * Pad tokens point at table row 0 and carry dstslot=-1, so their
    one-hot column is zero and they contribute nothing.
"""

import os
import sys

import numpy as np

for _p in ("/opt/trn_rl_repo", "/root/.axon_site/_ro/trn_rl_repo"):
    if os.path.isdir(_p) and _p not in sys.path:
        sys.path.insert(0, _p)

# ---------------------------------------------------------------- constants
N = 50000
E = 800000
IN_DIM = 128
HID = 16
HEADS = 8
OUT_DIM = 32
NEG_SLOPE = 0.2

CORES = 8
NPC = N // CORES          # nodes per core
P = 128
SPLIT = 32768             # table-A rows (int16 index limit)
NG = (NPC + P - 1) // P   # dst groups per core (49)
GTOK = 1024               # tokens per dma_gather (ring cap ~1.5k)
GCOLS = GTOK // P         # 8 columns per gather
NQ = 4                    # SWDGE queues
DENSE_W = 144
NT = (NPC + P - 1) // P

_PLAN_CACHE = {}


def _bass_mods():
    import concourse.bass as bass
    import concourse.tile as tile
    from concourse import mybir

    return bass, tile, mybir


_SAFE_TC = None


def _safe_tile_context():
    """TileContext whose kernel-tail drain never carries more than 2 sem
    waits per instruction (this container's walrus rejects >2 sync-wait
    commands on the SP CTRL drain); excess waits are moved onto preceding
    SP nops."""
    global _SAFE_TC
    if _SAFE_TC is not None:
        return _SAFE_TC
    import concourse.tile as tile
    from concourse import mybir
    from concourse.vector_clock import ScopedClock

    class TileContextSafe(tile.TileContext):
        def _add_instruction(self, inst):
            si = inst.sync_info
            if (
                si is not None
                and si.on_wait
                and len(si.on_wait) > 1
                and inst.engine != mybir.EngineType.Unassigned
            ):
                waits = list(si.on_wait)
                si.on_wait = waits[-1:]
                for w in waits[:-1]:
                    nop = mybir.InstNoOp(
                        name=self.nc.get_next_instruction_name(), ins=[], outs=[]
                    )
                    nop.engine = inst.engine
                    nop.sync_info = mybir.SyncInfo(on_wait=[w], on_update=[])
                    super()._add_instruction(nop)
            super()._add_instruction(inst)

        def _drain_and_barrier(self, tick_clock, wait_clock):
            nc = self.nc
            nops = [nc.sync.nop(nofuse=True) for _ in range(28)]
            drain_inst = nc.sync.drain()
            wait_clock.add_sem_waits(
                drain_inst.ins, ScopedClock({None: tick_clock.global_clock})
            )
            si = drain_inst.ins.sync_info
            waits = list(si.on_wait) if si is not None and si.on_wait else []
            if len(waits) > 1:
                si.on_wait = waits[:1]
                rest = waits[1:]
                assert len(rest) <= len(nops), "raise nop count"
                for k, w in enumerate(rest):
                    nops[k].ins.sync_info = mybir.SyncInfo(
                        on_wait=[w], on_update=[]
                    )

            nc.all_engine_barrier()
            assert self.sems is not None
            popped = nc._tile_sem_poison_stack.pop()
            assert popped is self._sem_poison
            nc.clear_and_free_semaphores(list(self.sems.allocated().values()))
            nc.all_engine_barrier()

    _SAFE_TC = TileContextSafe
    return _SAFE_TC


def _ap(tile_ap, col_off, dims):
    import concourse.bass as bass

    part = list(tile_ap.ap[0])
    return bass.AP(
        tile_ap.tensor,
        tile_ap.offset + col_off,
        [part] + [list(d) for d in dims],
    )


# ---------------------------------------------------------------- host prep
def fuse_weights(W, a_src, a_dst, H, D):
    """W:[K, H*D] -> [K, DENSE_W] = [W | Wes | Wed] (zero padded)."""
    K = W.shape[0]
    Wr = W.reshape(K, H, D)
    wes = np.einsum("khd,hd->kh", Wr, a_src)
    wed = np.einsum("khd,hd->kh", Wr, a_dst)
    out = np.zeros((K, DENSE_W), dtype=np.float32)
    out[:, : H * D] = W
    out[:, H * D : H * D + H] = wes
    out[:, H * D + H : H * D + 2 * H] = wed
    return out


def pack_idx16(tok):
    """[T] int -> [128, T//16] int16; token t at [t%16, t//16], replicated
    across the 8 groups of 16 partitions."""
    T = len(tok)
    assert T % 16 == 0
    a = np.asarray(tok, dtype=np.int16).reshape(T // 16, 16).T  # [16, T/16]
    return np.tile(a, (8, 1))


def build_plan_v4(src, dst, n=N, cores=CORES, npc=NPC, split=SPLIT):
    """Token streams for the v4 edge kernel (layer-independent).

    Static (shared across cores): ncA/ncB columns per group, gather
    chunk list.  Per core: int16 index streams, dstslot array.
    """
    key = ("v4", src.tobytes(), dst.tobytes(), n, cores, npc, split)
    h = hash(key)
    if h in _PLAN_CACHE:
        return _PLAN_CACHE[h]

    ng = (npc + P - 1) // P
    order = np.argsort(dst, kind="stable")
    ssrc = src[order].astype(np.int64)
    sdst = dst[order].astype(np.int64)
    core_of = sdst // npc
    deg = np.bincount(dst, minlength=n).astype(np.int64)
    starts = np.zeros(n + 1, dtype=np.int64)
    np.cumsum(deg, out=starts[1:])

    # per (core, group): A/B edge lists sorted by src
    eA = [[None] * ng for _ in range(cores)]
    eB = [[None] * ng for _ in range(cores)]
    for c in range(cores):
        base = c * npc
        for g in range(ng):
            lo = base + g * P
            hi = min(base + (g + 1) * P, base + npc)
            es_ = ssrc[starts[lo] : starts[hi]]
            ds_ = sdst[starts[lo] : starts[hi]]
            o = np.argsort(es_, kind="stable")
            es_, ds_ = es_[o], ds_[o]
            half = np.searchsorted(es_, split)
            eA[c][g] = (es_[:half], ds_[:half] - lo)
            eB[c][g] = (es_[half:] - split, ds_[half:] - lo)

    ncA = [
        max(1, max((len(eA[c][g][0]) + P - 1) // P for c in range(cores)))
        for g in range(ng)
    ]
    ncB = [
        max(1, max((len(eB[c][g][0]) + P - 1) // P for c in range(cores)))
        for g in range(ng)
    ]
    CA, CB = sum(ncA), sum(ncB)

    cores_arr = []
    for c in range(cores):
        tokA = np.zeros(CA * P, dtype=np.int16)
        tokB = np.zeros(CB * P, dtype=np.int16)
        import ml_dtypes
        dsl = np.full((P, CA + CB), -1.0, dtype=ml_dtypes.bfloat16)
        for phase, (toks, ncX, eX, coff) in enumerate(
            (
                (tokA, ncA, eA, 0),
                (tokB, ncB, eB, CA),
            )
        ):
            t0 = 0
            col = coff
            for g in range(ng):
                es_, dslot = eX[c][g]
                ne = len(es_)
                toks[t0 : t0 + ne] = es_.astype(np.int16)
                j = np.arange(ne)
                dsl[j % P, col + j // P] = dslot.astype(ml_dtypes.bfloat16)
                t0 += ncX[g] * P
                col += ncX[g]
        dslf = dsl.astype(np.float32)
        oh = (dslf[:, :, None] == np.arange(P, dtype=np.float32)[None, None, :])
        oh = np.ascontiguousarray(
            oh.reshape(P, (CA + CB) * P)).astype(ml_dtypes.bfloat16)
        gcol = np.zeros(CA + CB, dtype=np.int64)   # group of each column
        col = 0
        for ph, ncX in ((0, ncA), (1, ncB)):
            for g in range(ng):
                gcol[col : col + ncX[g]] = g
                col += ncX[g]
        tokdst = np.where(
            dslf >= 0, gcol[None, :] * P + dslf, -1.0
        ).astype(np.int64)
        tsA = np.zeros(CA * P, dtype=np.int64)
        tsB = np.zeros(CB * P, dtype=np.int64)
        for toks2, ncX, eX, off2 in (
            (tsA, ncA, eA, 0), (tsB, ncB, eB, split)
        ):
            t0b = 0
            for g in range(ng):
                es2, _ = eX[c][g]
                toks2[t0b : t0b + len(es2)] = es2 + off2
                t0b += ncX[g] * P
        allt = np.concatenate([tsA, tsB])
        toksrc = np.zeros((P, CA + CB), dtype=np.int64)
        tt = np.arange(len(allt))
        toksrc[tt % P, tt // P] = allt
        cores_arr.append(
            {
                "idxA": pack_idx16(tokA),
                "idxB": pack_idx16(tokB),
                "dsl": dsl,
                "oh": oh,
                "tokdst": tokdst,
                "toksrc": toksrc,
            }
        )

    # gather chunks: (phase, token_start_in_stream, ntok)
    chunks = []
    for phase, CX in ((0, CA), (1, CB)):
        t = 0
        while t < CX * P:
            nt = min(GTOK, CX * P - t)
            chunks.append((phase, t, nt))
            t += nt

    plan = {
        "ng": ng,
        "ncA": ncA,
        "ncB": ncB,
        "CA": CA,
        "CB": CB,
        "chunks": chunks,
        "cores": cores_arr,
    }
    tok_tot = (CA + CB) * P
    edge_tot = sum(len(eA[c][g][0]) + len(eB[c][g][0])
                   for c in range(cores) for g in range(ng)) / cores
    plan["pad_frac"] = tok_tot / max(edge_tot, 1) - 1.0
    _PLAN_CACHE[h] = plan
    return plan


# ---------------------------------------------------------------- edge nc
def build_edge_nc_v4(plan, RWE, H, D, elu, n=N, npc=NPC, split=SPLIT):
    """Edge kernel for one GAT layer (one program, SPMD over cores).

    T    [n, RWE]  bf16  row = [z (H*D) | es (H) | pad]
    IDXA [128, CA*8] i16; IDXB [128, CB*8] i16
    DSL  [128, CA+CB] f32   dst slot per token (-1 = pad)
    EDC  [ng*128, H] bf16   ed rows of this core's nodes
    IOTA [128, 128] bf16    iota[p, j] = j
    IDENT[128, 128] bf16    identity
    OUT  [ng*128, H*D] f32
    """
    bass, tile, mybir = _bass_mods()
    from contextlib import ExitStack
    from concourse.library_config import mlp

    f32 = mybir.dt.float32
    bf16 = mybir.dt.bfloat16
    i16 = mybir.dt.int16

    ZW = H * D
    MW = ZW + H
    ng = plan["ng"]
    ncA, ncB = plan["ncA"], plan["ncB"]
    CA, CB = plan["CA"], plan["CB"]
    chunks = plan["chunks"]

    nc = bass.Bass("TRN2", num_swdge_queues=NQ)
    T = nc.dram_tensor("tbl", [n, RWE], bf16, kind="ExternalInput")
    IDXA = nc.dram_tensor("idxa", [P, CA * 8], i16, kind="ExternalInput")
    IDXB = nc.dram_tensor("idxb", [P, CB * 8], i16, kind="ExternalInput")
    DSL = nc.dram_tensor("dsl", [P, CA + CB], bf16, kind="ExternalInput")
    EDC = nc.dram_tensor("edc", [ng * P, H], bf16, kind="ExternalInput")
    IOTA = nc.dram_tensor("iota", [P, P], bf16, kind="ExternalInput")
    IDENT = nc.dram_tensor("ident", [P, P], bf16, kind="ExternalInput")
    OUT = nc.dram_tensor("out", [ng * P, ZW], f32, kind="ExternalOutput")

    # column -> (phase, group, first?, last?) map
    colmap = []
    for phase, ncX in ((0, ncA), (1, ncB)):
        for g in range(ng):
            for k in range(ncX[g]):
                colmap.append((phase, g, k == 0, k == ncX[g] - 1))

    with _safe_tile_context()(nc) as tc:
        with ExitStack() as ctx:
            nc.gpsimd.load_library(mlp)
            const = ctx.enter_context(tc.tile_pool(name="const", bufs=1))
            gath = ctx.enter_context(tc.tile_pool(name="gath", bufs=6))
            meta = ctx.enter_context(tc.tile_pool(name="meta", bufs=2))
            work = ctx.enter_context(tc.tile_pool(name="work", bufs=3))
            psum = ctx.enter_context(
                tc.tile_pool(name="psum", bufs=2, space="PSUM")
            )
            psT = ctx.enter_context(
                tc.tile_pool(name="psT", bufs=3, space="PSUM")
            )

            iota = const.tile([P, P], bf16)
            nc.sync.dma_start(out=iota[:], in_=IOTA[:, :])
            ident = const.tile([P, P], bf16)
            nc.sync.dma_start(out=ident[:], in_=IDENT[:, :])
            idxa = const.tile([P, CA * 8], i16)
            nc.sync.dma_start(out=idxa[:], in_=IDXA[:, :])
            idxb = const.tile([P, CB * 8], i16)
            nc.sync.dma_start(out=idxb[:], in_=IDXB[:, :])
            dsl = const.tile([P, CA + CB], bf16)
            nc.sync.dma_start(out=dsl[:], in_=DSL[:, :])
            park = const.tile([P, ng * MW], f32)
            edg_all = const.tile([P, ng * H], bf16)
            for g in range(ng):
                nc.sync.dma_start(
                    out=edg_all[:, g * H : (g + 1) * H],
                    in_=EDC[g * P : (g + 1) * P, :],
                )

            # pre-zero the rotating gather buffers (stale SBUF may be NaN;
            # pad-token garbage must stay finite)
            gbufs = []
            for _ in range(6):
                gb = gath.tile([P, GCOLS * RWE], bf16, tag="gt")
                nc.vector.memset(gb[:], 0.0)
                gbufs.append(gb)

            ni_reg = nc.gpsimd.to_reg(GTOK)

            agg_ps = None   # current group's PSUM agg tile
            ci = 0          # global column index
            for gi, (phase, t0, nt) in enumerate(chunks):
              idxt = idxa if phase == 0 else idxb
              gb = gath.tile([P, GCOLS * RWE], bf16, tag="gt")
              src_t = T[:split, :] if phase == 0 else T[split:, :]
              reg = ni_reg if nt == GTOK else nt
              nc.gpsimd.dma_gather(
                  _ap(gb[:], 0, [[RWE, (nt + P - 1) // P], [1, RWE]]),
                  src_t,
                  idxt[:, t0 // 16 : t0 // 16 + (nt + 15) // 16],
                  nt,
                  reg,
                  RWE,
                  queue_num=gi % NQ,
              )
              for slot in range(nt // P):
                c = ci
                ci += 1
                phase_c, g, first, last = colmap[c]
                assert phase_c == phase
                gbase = slot * RWE

                # one-hot oh[t, d]
                oh = work.tile([P, P], bf16, tag="oh")
                nc.vector.tensor_tensor(
                    out=oh[:],
                    in0=_ap(dsl[:], c, [[0, P]]),
                    in1=iota[:],
                    op=mybir.AluOpType.is_equal,
                )
                # ohT via PE transpose
                pst = psT.tile([P, P], f32, tag="ohT")
                nc.tensor.matmul(
                    out=pst[:], lhsT=oh[:], rhs=ident[:], start=True, stop=True
                )
                ohT = work.tile([P, P], bf16, tag="ohTs")
                nc.scalar.activation(
                    out=ohT[:], in_=pst[:],
                    func=mybir.ActivationFunctionType.Copy,
                )
                # per-token ed
                psed = psT.tile([P, H], f32, tag="ed")
                nc.tensor.matmul(
                    out=psed[:], lhsT=ohT[:],
                    rhs=edg_all[:, g * H : (g + 1) * H],
                    start=True, stop=True,
                )
                # w = exp(leakyrelu(es + ed))
                es_f = work.tile([P, H], f32, tag="esf")
                nc.vector.tensor_copy(
                    out=es_f[:], in_=_ap(gb[:], gbase + ZW, [[1, H]])
                )
                e_t = work.tile([P, H], f32, tag="e")
                nc.vector.tensor_tensor(
                    out=e_t[:],
                    in0=es_f[:],
                    in1=psed[:],
                    op=mybir.AluOpType.add,
                )
                t2 = work.tile([P, H], f32, tag="t2")
                nc.vector.tensor_scalar_mul(t2[:], e_t[:], NEG_SLOPE)
                t3 = work.tile([P, H], f32, tag="t3")
                nc.vector.tensor_tensor(
                    out=t3[:], in0=e_t[:], in1=t2[:], op=mybir.AluOpType.max
                )
                w_t = work.tile([P, H], bf16, tag="w")
                nc.scalar.activation(
                    out=w_t[:], in_=t3[:], func=mybir.ActivationFunctionType.Exp
                )
                # m = [w*z | w]
                m_t = work.tile([P, MW], bf16, tag="m")
                nc.vector.tensor_tensor(
                    out=_ap(m_t[:], 0, [[1, ZW]]),
                    in0=_ap(gb[:], gbase, [[1, ZW]]),
                    in1=_ap(w_t[:], 0, [[1, H], [0, D]]),
                    op=mybir.AluOpType.mult,
                )
                nc.vector.tensor_copy(out=_ap(m_t[:], ZW, [[1, H]]), in_=w_t[:])
                # aggregate
                if first:
                    agg_ps = psum.tile([P, MW], f32, tag="agg")
                nc.tensor.matmul(
                    out=agg_ps[:], lhsT=oh[:], rhs=m_t[:],
                    start=first, stop=last,
                )
                if last and phase == 0:
                    nc.vector.tensor_copy(
                        out=_ap(park[:], g * MW, [[1, MW]]), in_=agg_ps[:]
                    )
                if last and phase == 1:
                    tot = work.tile([P, MW], f32, tag="tot")
                    nc.vector.tensor_tensor(
                        out=tot[:],
                        in0=agg_ps[:],
                        in1=_ap(park[:], g * MW, [[1, MW]]),
                        op=mybir.AluOpType.add,
                    )
                    sden = work.tile([P, H], f32, tag="sden")
                    nc.vector.tensor_scalar_add(sden[:], tot[:, ZW:MW], 1e-30)
                    rs = work.tile([P, H], f32, tag="rs")
                    nc.vector.reciprocal(rs[:], sden[:])
                    h1 = work.tile([P, ZW], f32, tag="h1")
                    nc.vector.tensor_tensor(
                        out=h1[:],
                        in0=tot[:, :ZW],
                        in1=_ap(rs[:], 0, [[1, H], [0, D]]),
                        op=mybir.AluOpType.mult,
                    )
                    if elu:
                        pos = work.tile([P, ZW], f32, tag="pos")
                        nc.vector.tensor_scalar_max(pos[:], h1[:], 0.0)
                        ngx = work.tile([P, ZW], f32, tag="ngx")
                        nc.vector.tensor_scalar_min(ngx[:], h1[:], 0.0)
                        ex = work.tile([P, ZW], f32, tag="ex")
                        nc.scalar.activation(
                            out=ex[:], in_=ngx[:],
                            func=mybir.ActivationFunctionType.Exp,
                        )
                        hf = work.tile([P, ZW], f32, tag="hf")
                        nc.vector.tensor_tensor(
                            out=hf[:], in0=pos[:], in1=ex[:],
                            op=mybir.AluOpType.add,
                        )
                        out_t = work.tile([P, ZW], f32, tag="outt")
                        nc.vector.tensor_scalar_add(out_t[:], hf[:], -1.0)
                    else:
                        out_t = h1
                    nc.sync.dma_start(
                        out=OUT[g * P : (g + 1) * P, :], in_=out_t[:]
                    )
    mybir.codegen_inst_isa_subclasses(nc)
    return nc




# ------------------------------------------------- edge nc (batched DVE)
def build_edge_nc_v4b(plan, RWE, H, D, elu, n=N, npc=NPC, split=SPLIT):
    """Like build_edge_nc_v4 but DVE/ACT work batched per gather buffer
    (up to GCOLS columns per instruction) to amortize instruction
    dispatch overhead.  PE matmuls stay per column."""
    bass, tile, mybir = _bass_mods()
    from contextlib import ExitStack
    from concourse.library_config import mlp

    f32 = mybir.dt.float32
    bf16 = mybir.dt.bfloat16
    i16 = mybir.dt.int16

    ZW = H * D
    MW = ZW + H
    ng = plan["ng"]
    ncA, ncB = plan["ncA"], plan["ncB"]
    CA, CB = plan["CA"], plan["CB"]
    chunks = plan["chunks"]

    nc = bass.Bass("TRN2", num_swdge_queues=NQ)
    T = nc.dram_tensor("tbl", [n, RWE], bf16, kind="ExternalInput")
    IDXA = nc.dram_tensor("idxa", [P, CA * 8], i16, kind="ExternalInput")
    IDXB = nc.dram_tensor("idxb", [P, CB * 8], i16, kind="ExternalInput")
    DSL = nc.dram_tensor("dsl", [P, CA + CB], bf16, kind="ExternalInput")
    EDC = nc.dram_tensor("edc", [ng * P, H], bf16, kind="ExternalInput")
    IOTA = nc.dram_tensor("iota", [P, P], bf16, kind="ExternalInput")
    IDENT = nc.dram_tensor("ident", [P, P], bf16, kind="ExternalInput")
    OUT = nc.dram_tensor("out", [ng * P, ZW], f32, kind="ExternalOutput")

    colmap = []
    for phase, ncX in ((0, ncA), (1, ncB)):
        for g in range(ng):
            for k in range(ncX[g]):
                colmap.append((phase, g, k == 0, k == ncX[g] - 1))

    with _safe_tile_context()(nc) as tc:
        with ExitStack() as ctx:
            nc.gpsimd.load_library(mlp)
            const = ctx.enter_context(tc.tile_pool(name="const", bufs=1))
            gath = ctx.enter_context(tc.tile_pool(name="gath", bufs=6))
            work = ctx.enter_context(tc.tile_pool(name="work", bufs=3))
            psum = ctx.enter_context(
                tc.tile_pool(name="psum", bufs=2, space="PSUM")
            )
            psT = ctx.enter_context(
                tc.tile_pool(name="psT", bufs=3, space="PSUM")
            )

            iota = const.tile([P, P], bf16)
            nc.sync.dma_start(out=iota[:], in_=IOTA[:, :])
            ident = const.tile([P, P], bf16)
            nc.sync.dma_start(out=ident[:], in_=IDENT[:, :])
            idxa = const.tile([P, CA * 8], i16)
            nc.sync.dma_start(out=idxa[:], in_=IDXA[:, :])
            idxb = const.tile([P, CB * 8], i16)
            nc.sync.dma_start(out=idxb[:], in_=IDXB[:, :])
            dsl = const.tile([P, CA + CB], bf16)
            nc.sync.dma_start(out=dsl[:], in_=DSL[:, :])
            park = const.tile([P, ng * MW], f32)
            edg_all = const.tile([P, ng * H], bf16)
            for g in range(ng):
                nc.sync.dma_start(
                    out=edg_all[:, g * H : (g + 1) * H],
                    in_=EDC[g * P : (g + 1) * P, :],
                )

            for _ in range(6):
                gb = gath.tile([P, GCOLS * RWE], bf16, tag="gt")
                nc.vector.memset(gb[:], 0.0)

            ni_reg = nc.gpsimd.to_reg(GTOK)

            agg_ps = None
            ci = 0
            for gi, (phase, t0, nt) in enumerate(chunks):
                W8 = nt // P          # columns in this buffer
                idxt = idxa if phase == 0 else idxb
                gb = gath.tile([P, GCOLS * RWE], bf16, tag="gt")
                src_t = T[:split, :] if phase == 0 else T[split:, :]
                reg = ni_reg if nt == GTOK else nt
                nc.gpsimd.dma_gather(
                    _ap(gb[:], 0, [[RWE, W8], [1, RWE]]),
                    src_t,
                    idxt[:, t0 // 16 : t0 // 16 + nt // 16],
                    nt,
                    reg,
                    RWE,
                    queue_num=gi % NQ,
                )

                # ---- batched per-buffer work
                oh_b = work.tile([P, GCOLS * P], bf16, tag="oh")
                nc.vector.tensor_tensor(
                    out=_ap(oh_b[:], 0, [[P, W8], [1, P]]),
                    in0=_ap(dsl[:], ci, [[1, W8], [0, P]]),
                    in1=_ap(iota[:], 0, [[0, W8], [1, P]]),
                    op=mybir.AluOpType.is_equal,
                )
                # per-token ed via PE transpose + matmul (per column)
                psed_b = psT.tile([P, GCOLS * H], f32, tag="ed")
                ohT_b = work.tile([P, GCOLS * P], bf16, tag="ohTs")
                for j in range(W8):
                    pst = psT.tile([P, P], f32, tag="ohT")
                    nc.tensor.matmul(
                        out=pst[:],
                        lhsT=oh_b[:, j * P : (j + 1) * P],
                        rhs=ident[:],
                        start=True,
                        stop=True,
                    )
                    nc.scalar.activation(
                        out=ohT_b[:, j * P : (j + 1) * P], in_=pst[:],
                        func=mybir.ActivationFunctionType.Copy,
                    )
                    _, g, _, _ = colmap[ci + j]
                    nc.tensor.matmul(
                        out=psed_b[:, j * H : (j + 1) * H],
                        lhsT=ohT_b[:, j * P : (j + 1) * P],
                        rhs=edg_all[:, g * H : (g + 1) * H],
                        start=True,
                        stop=True,
                    )
                # w = exp(leakyrelu(es + ed)), batched
                es_f = work.tile([P, GCOLS * H], f32, tag="esf")
                nc.vector.tensor_copy(
                    out=_ap(es_f[:], 0, [[H, W8], [1, H]]),
                    in_=_ap(gb[:], ZW, [[RWE, W8], [1, H]]),
                )
                e_t = work.tile([P, GCOLS * H], f32, tag="e")
                nc.vector.tensor_tensor(
                    out=_ap(e_t[:], 0, [[H, W8], [1, H]]),
                    in0=_ap(es_f[:], 0, [[H, W8], [1, H]]),
                    in1=_ap(psed_b[:], 0, [[H, W8], [1, H]]),
                    op=mybir.AluOpType.add,
                )
                EC = W8 * H
                t2 = work.tile([P, GCOLS * H], f32, tag="t2")
                nc.vector.tensor_scalar_mul(t2[:, :EC], e_t[:, :EC], NEG_SLOPE)
                t3 = work.tile([P, GCOLS * H], f32, tag="t3")
                nc.vector.tensor_tensor(
                    out=t3[:, :EC], in0=e_t[:, :EC], in1=t2[:, :EC],
                    op=mybir.AluOpType.max,
                )
                w_b = work.tile([P, GCOLS * H], bf16, tag="w")
                nc.scalar.activation(
                    out=w_b[:, :EC], in_=t3[:, :EC],
                    func=mybir.ActivationFunctionType.Exp,
                )
                # m = [w*z | w], batched
                m_b = work.tile([P, GCOLS * MW], bf16, tag="m")
                nc.vector.tensor_tensor(
                    out=_ap(m_b[:], 0, [[MW, W8], [1, ZW]]),
                    in0=_ap(gb[:], 0, [[RWE, W8], [1, ZW]]),
                    in1=_ap(w_b[:], 0, [[H, W8], [1, H], [0, D]]),
                    op=mybir.AluOpType.mult,
                )
                nc.vector.tensor_copy(
                    out=_ap(m_b[:], ZW, [[MW, W8], [1, H]]),
                    in_=w_b[:, :EC],
                )
                # ---- aggregation + epilogue per column
                for j in range(W8):
                    c = ci + j
                    _, g, first, last = colmap[c]
                    if first:
                        agg_ps = psum.tile([P, MW], f32, tag="agg")
                    nc.tensor.matmul(
                        out=agg_ps[:],
                        lhsT=oh_b[:, j * P : (j + 1) * P],
                        rhs=m_b[:, j * MW : (j + 1) * MW],
                        start=first,
                        stop=last,
                    )
                    if last and phase == 0:
                        nc.vector.tensor_copy(
                            out=_ap(park[:], g * MW, [[1, MW]]), in_=agg_ps[:]
                        )
                    if last and phase == 1:
                        tot = work.tile([P, MW], f32, tag="tot")
                        nc.vector.tensor_tensor(
                            out=tot[:],
                            in0=agg_ps[:],
                            in1=_ap(park[:], g * MW, [[1, MW]]),
                            op=mybir.AluOpType.add,
                        )
                        sden = work.tile([P, H], f32, tag="sden")
                        nc.vector.tensor_scalar_add(
                            sden[:], tot[:, ZW:MW], 1e-30
                        )
                        rs = work.tile([P, H], f32, tag="rs")
                        nc.vector.reciprocal(rs[:], sden[:])
                        h1 = work.tile([P, ZW], f32, tag="h1")
                        nc.vector.tensor_tensor(
                            out=h1[:],
                            in0=tot[:, :ZW],
                            in1=_ap(rs[:], 0, [[1, H], [0, D]]),
                            op=mybir.AluOpType.mult,
                        )
                        if elu:
                            pos = work.tile([P, ZW], f32, tag="pos")
                            nc.vector.tensor_scalar_max(pos[:], h1[:], 0.0)
                            ngx = work.tile([P, ZW], f32, tag="ngx")
                            nc.vector.tensor_scalar_min(ngx[:], h1[:], 0.0)
                            ex = work.tile([P, ZW], f32, tag="ex")
                            nc.scalar.activation(
                                out=ex[:], in_=ngx[:],
                                func=mybir.ActivationFunctionType.Exp,
                            )
                            hf = work.tile([P, ZW], f32, tag="hf")
                            nc.vector.tensor_tensor(
                                out=hf[:], in0=pos[:], in1=ex[:],
                                op=mybir.AluOpType.add,
                            )
                            out_t = work.tile([P, ZW], f32, tag="outt")
                            nc.vector.tensor_scalar_add(out_t[:], hf[:], -1.0)
                        else:
                            out_t = h1
                        nc.sync.dma_start(
                            out=OUT[g * P : (g + 1) * P, :], in_=out_t[:]
                        )
                ci += W8
    mybir.codegen_inst_isa_subclasses(nc)
    return nc


# ------------------------------------------------- edge nc v5 (host oh/edt)
def build_edge_nc_v5(plan, RWE, H, D, elu, n=N, npc=NPC, split=SPLIT):
    """v4b minus on-device one-hot build and per-token-ed machinery.

    OHS [128, ncols*128] bf16  host-built one-hot (streamed per buffer)
    EDT [128, ncols*H]   bf16  host-gathered ed of each token's dst
    """
    bass, tile, mybir = _bass_mods()
    from contextlib import ExitStack
    from concourse.library_config import mlp

    f32 = mybir.dt.float32
    bf16 = mybir.dt.bfloat16
    i16 = mybir.dt.int16

    ZW = H * D
    MW = ZW + H
    ng = plan["ng"]
    ncA, ncB = plan["ncA"], plan["ncB"]
    CA, CB = plan["CA"], plan["CB"]
    chunks = plan["chunks"]
    NC = CA + CB

    nc = bass.Bass("TRN2", num_swdge_queues=NQ)
    T = nc.dram_tensor("tbl", [n, RWE], bf16, kind="ExternalInput")
    IDXA = nc.dram_tensor("idxa", [P, CA * 8], i16, kind="ExternalInput")
    IDXB = nc.dram_tensor("idxb", [P, CB * 8], i16, kind="ExternalInput")
    OHS = nc.dram_tensor("ohs", [P, NC * P], bf16, kind="ExternalInput")
    EDT = nc.dram_tensor("edt", [P, NC * H], bf16, kind="ExternalInput")
    OUT = nc.dram_tensor("out", [ng * P, ZW], f32, kind="ExternalOutput")

    colmap = []
    for phase, ncX in ((0, ncA), (1, ncB)):
        for g in range(ng):
            for k in range(ncX[g]):
                colmap.append((phase, g, k == 0, k == ncX[g] - 1))

    with _safe_tile_context()(nc) as tc:
        with ExitStack() as ctx:
            nc.gpsimd.load_library(mlp)
            const = ctx.enter_context(tc.tile_pool(name="const", bufs=1))
            gath = ctx.enter_context(tc.tile_pool(name="gath", bufs=6))
            ohp = ctx.enter_context(tc.tile_pool(name="ohp", bufs=6))
            work = ctx.enter_context(tc.tile_pool(name="work", bufs=3))
            psum = ctx.enter_context(
                tc.tile_pool(name="psum", bufs=2, space="PSUM")
            )

            idxa = const.tile([P, CA * 8], i16)
            nc.sync.dma_start(out=idxa[:], in_=IDXA[:, :])
            idxb = const.tile([P, CB * 8], i16)
            nc.sync.dma_start(out=idxb[:], in_=IDXB[:, :])
            edt = const.tile([P, NC * H], bf16)
            nc.sync.dma_start(out=edt[:], in_=EDT[:, :])
            park = const.tile([P, ng * MW], f32)

            for _ in range(6):
                gb = gath.tile([P, GCOLS * RWE], bf16, tag="gt")
                nc.vector.memset(gb[:], 0.0)

            ni_reg = nc.gpsimd.to_reg(GTOK)

            agg_ps = None
            ci = 0
            for gi, (phase, t0, nt) in enumerate(chunks):
                W8 = nt // P
                idxt = idxa if phase == 0 else idxb
                gb = gath.tile([P, GCOLS * RWE], bf16, tag="gt")
                src_t = T[:split, :] if phase == 0 else T[split:, :]
                reg = ni_reg if nt == GTOK else nt
                nc.gpsimd.dma_gather(
                    _ap(gb[:], 0, [[RWE, W8], [1, RWE]]),
                    src_t,
                    idxt[:, t0 // 16 : t0 // 16 + nt // 16],
                    nt,
                    reg,
                    RWE,
                    queue_num=gi % NQ,
                )
                oh_b = ohp.tile([P, GCOLS * P], bf16, tag="oh")
                nc.sync.dma_start(
                    out=oh_b[:, : W8 * P],
                    in_=OHS[:, ci * P : (ci + W8) * P],
                )

                # w = exp(leakyrelu(es + ed)), batched per buffer
                e_t = work.tile([P, GCOLS * H], f32, tag="e")
                nc.vector.tensor_tensor(
                    out=_ap(e_t[:], 0, [[H, W8], [1, H]]),
                    in0=_ap(gb[:], ZW, [[RWE, W8], [1, H]]),
                    in1=_ap(edt[:], ci * H, [[H, W8], [1, H]]),
                    op=mybir.AluOpType.add,
                )
                EC = W8 * H
                t2 = work.tile([P, GCOLS * H], f32, tag="t2")
                nc.vector.tensor_scalar_mul(t2[:, :EC], e_t[:, :EC], NEG_SLOPE)
                t3 = work.tile([P, GCOLS * H], f32, tag="t3")
                nc.vector.tensor_tensor(
                    out=t3[:, :EC], in0=e_t[:, :EC], in1=t2[:, :EC],
                    op=mybir.AluOpType.max,
                )
                w_b = work.tile([P, GCOLS * H], bf16, tag="w")
                nc.scalar.activation(
                    out=w_b[:, :EC], in_=t3[:, :EC],
                    func=mybir.ActivationFunctionType.Exp,
                )
                m_b = work.tile([P, GCOLS * MW], bf16, tag="m")
                nc.vector.tensor_tensor(
                    out=_ap(m_b[:], 0, [[MW, W8], [1, ZW]]),
                    in0=_ap(gb[:], 0, [[RWE, W8], [1, ZW]]),
                    in1=_ap(w_b[:], 0, [[H, W8], [1, H], [0, D]]),
                    op=mybir.AluOpType.mult,
                )
                nc.vector.tensor_copy(
                    out=_ap(m_b[:], ZW, [[MW, W8], [1, H]]),
                    in_=w_b[:, :EC],
                )
                for j in range(W8):
                    c = ci + j
                    _, g, first, last = colmap[c]
                    if first:
                        agg_ps = psum.tile([P, MW], f32, tag="agg")
                    nc.tensor.matmul(
                        out=agg_ps[:],
                        lhsT=oh_b[:, j * P : (j + 1) * P],
                        rhs=m_b[:, j * MW : (j + 1) * MW],
                        start=first,
                        stop=last,
                    )
                    if last and phase == 0:
                        nc.vector.tensor_copy(
                            out=_ap(park[:], g * MW, [[1, MW]]), in_=agg_ps[:]
                        )
                    if last and phase == 1:
                        tot = work.tile([P, MW], f32, tag="tot")
                        nc.vector.tensor_tensor(
                            out=tot[:],
                            in0=agg_ps[:],
                            in1=_ap(park[:], g * MW, [[1, MW]]),
                            op=mybir.AluOpType.add,
                        )
                        sden = work.tile([P, H], f32, tag="sden")
                        nc.vector.tensor_scalar_add(
                            sden[:], tot[:, ZW:MW], 1e-30
                        )
                        rs = work.tile([P, H], f32, tag="rs")
                        nc.vector.reciprocal(rs[:], sden[:])
                        h1 = work.tile([P, ZW], f32, tag="h1")
                        nc.vector.tensor_tensor(
                            out=h1[:],
                            in0=tot[:, :ZW],
                            in1=_ap(rs[:], 0, [[1, H], [0, D]]),
                            op=mybir.AluOpType.mult,
                        )
                        if elu:
                            pos = work.tile([P, ZW], f32, tag="pos")
                            nc.vector.tensor_scalar_max(pos[:], h1[:], 0.0)
                            ngx = work.tile([P, ZW], f32, tag="ngx")
                            nc.vector.tensor_scalar_min(ngx[:], h1[:], 0.0)
                            ex = work.tile([P, ZW], f32, tag="ex")
                            nc.scalar.activation(
                                out=ex[:], in_=ngx[:],
                                func=mybir.ActivationFunctionType.Exp,
                            )
                            hf = work.tile([P, ZW], f32, tag="hf")
                            nc.vector.tensor_tensor(
                                out=hf[:], in0=pos[:], in1=ex[:],
                                op=mybir.AluOpType.add,
                            )
                            out_t = work.tile([P, ZW], f32, tag="outt")
                            nc.vector.tensor_scalar_add(out_t[:], hf[:], -1.0)
                        else:
                            out_t = h1
                        nc.sync.dma_start(
                            out=OUT[g * P : (g + 1) * P, :], in_=out_t[:]
                        )
                ci += W8
    mybir.codegen_inst_isa_subclasses(nc)
    return nc


# ------------------------------------------------- edge nc v6 (z-only rows)
def build_edge_nc_v6(plan, RWE, H, D, elu, n=N, npc=NPC, split=SPLIT):
    """v5 with z-only gathered rows (es folded into the host-built
    e_tok = es[src]+ed[dst] array), contiguous message layout, and the
    denominator via a second small matmul.  `elu` is ignored here (ELU
    is applied by the next dense kernel)."""
    bass, tile, mybir = _bass_mods()
    from contextlib import ExitStack
    from concourse.library_config import mlp

    f32 = mybir.dt.float32
    bf16 = mybir.dt.bfloat16
    i16 = mybir.dt.int16

    ZW = H * D
    MW = ZW + H
    ng = plan["ng"]
    ncA, ncB = plan["ncA"], plan["ncB"]
    CA, CB = plan["CA"], plan["CB"]
    chunks = plan["chunks"]
    NC = CA + CB

    nc = bass.Bass("TRN2", num_swdge_queues=NQ)
    T = nc.dram_tensor("tbl", [n, RWE], bf16, kind="ExternalInput")
    IDXA = nc.dram_tensor("idxa", [P, CA * 8], i16, kind="ExternalInput")
    IDXB = nc.dram_tensor("idxb", [P, CB * 8], i16, kind="ExternalInput")
    OHS = nc.dram_tensor("ohs", [P, NC * P], bf16, kind="ExternalInput")
    ETOK = nc.dram_tensor("etok", [P, NC * H], bf16, kind="ExternalInput")
    OUT = nc.dram_tensor("out", [ng * P, ZW], f32, kind="ExternalOutput")

    colmap = []
    for phase, ncX in ((0, ncA), (1, ncB)):
        for g in range(ng):
            for k in range(ncX[g]):
                colmap.append((phase, g, k == 0, k == ncX[g] - 1))

    with _safe_tile_context()(nc) as tc:
        with ExitStack() as ctx:
            nc.gpsimd.load_library(mlp)
            const = ctx.enter_context(tc.tile_pool(name="const", bufs=1))
            gath = ctx.enter_context(tc.tile_pool(name="gath", bufs=6))
            ohp = ctx.enter_context(tc.tile_pool(name="ohp", bufs=6))
            work = ctx.enter_context(tc.tile_pool(name="work", bufs=3))
            psum = ctx.enter_context(
                tc.tile_pool(name="psum", bufs=2, space="PSUM")
            )

            idxa = const.tile([P, CA * 8], i16)
            nc.sync.dma_start(out=idxa[:], in_=IDXA[:, :])
            idxb = const.tile([P, CB * 8], i16)
            nc.sync.dma_start(out=idxb[:], in_=IDXB[:, :])
            etok = const.tile([P, NC * H], bf16)
            nc.sync.dma_start(out=etok[:], in_=ETOK[:, :])
            park = const.tile([P, ng * MW], f32)

            for _ in range(6):
                gb = gath.tile([P, GCOLS * RWE], bf16, tag="gt")
                nc.vector.memset(gb[:], 0.0)

            ni_reg = nc.gpsimd.to_reg(GTOK)

            agg_ps = None
            ci = 0
            for gi, (phase, t0, nt) in enumerate(chunks):
                W8 = nt // P
                idxt = idxa if phase == 0 else idxb
                gb = gath.tile([P, GCOLS * RWE], bf16, tag="gt")
                src_t = T[:split, :] if phase == 0 else T[split:, :]
                reg = ni_reg if nt == GTOK else nt
                nc.gpsimd.dma_gather(
                    _ap(gb[:], 0, [[RWE, W8], [1, RWE]]),
                    src_t,
                    idxt[:, t0 // 16 : t0 // 16 + nt // 16],
                    nt,
                    reg,
                    RWE,
                    queue_num=gi % NQ,
                )
                oh_b = ohp.tile([P, GCOLS * P], bf16, tag="oh")
                nc.sync.dma_start(
                    out=oh_b[:, : W8 * P],
                    in_=OHS[:, ci * P : (ci + W8) * P],
                )

                # w = exp(leakyrelu(e_tok)); all slices contiguous
                EC = W8 * H
                e_sl = etok[:, ci * H : ci * H + EC]
                t2 = work.tile([P, GCOLS * H], f32, tag="t2")
                nc.vector.tensor_scalar_mul(t2[:, :EC], e_sl, NEG_SLOPE)
                t3 = work.tile([P, GCOLS * H], f32, tag="t3")
                nc.vector.tensor_tensor(
                    out=t3[:, :EC], in0=e_sl, in1=t2[:, :EC],
                    op=mybir.AluOpType.max,
                )
                w_b = work.tile([P, GCOLS * H], bf16, tag="w")
                nc.scalar.activation(
                    out=w_b[:, :EC], in_=t3[:, :EC],
                    func=mybir.ActivationFunctionType.Exp,
                )
                # m = w*z, contiguous out (gb rows are pure z)
                m_b = work.tile([P, GCOLS * ZW], bf16, tag="m")
                nc.vector.tensor_tensor(
                    out=m_b[:, : W8 * ZW],
                    in0=_ap(gb[:], 0, [[RWE, W8], [1, ZW]]),
                    in1=_ap(w_b[:], 0, [[H, W8], [1, H], [0, D]]),
                    op=mybir.AluOpType.mult,
                )
                for j in range(W8):
                    c = ci + j
                    _, g, first, last = colmap[c]
                    if first:
                        agg_ps = psum.tile([P, ZW], f32, tag="agg")
                        den_ps = psum.tile([P, H], f32, tag="den")
                    nc.tensor.matmul(
                        out=agg_ps[:],
                        lhsT=oh_b[:, j * P : (j + 1) * P],
                        rhs=m_b[:, j * ZW : (j + 1) * ZW],
                        start=first,
                        stop=last,
                    )
                    nc.tensor.matmul(
                        out=den_ps[:],
                        lhsT=oh_b[:, j * P : (j + 1) * P],
                        rhs=w_b[:, j * H : (j + 1) * H],
                        start=first,
                        stop=last,
                    )
                    if last and phase == 0:
                        nc.vector.tensor_copy(
                            out=_ap(park[:], g * MW, [[1, ZW]]), in_=agg_ps[:]
                        )
                        nc.vector.tensor_copy(
                            out=_ap(park[:], g * MW + ZW, [[1, H]]),
                            in_=den_ps[:],
                        )
                    if last and phase == 1:
                        totz = work.tile([P, ZW], f32, tag="totz")
                        nc.vector.tensor_tensor(
                            out=totz[:],
                            in0=agg_ps[:],
                            in1=_ap(park[:], g * MW, [[1, ZW]]),
                            op=mybir.AluOpType.add,
                        )
                        totd = work.tile([P, H], f32, tag="totd")
                        nc.vector.tensor_tensor(
                            out=totd[:],
                            in0=den_ps[:],
                            in1=_ap(park[:], g * MW + ZW, [[1, H]]),
                            op=mybir.AluOpType.add,
                        )
                        sden = work.tile([P, H], f32, tag="sden")
                        nc.vector.tensor_scalar_add(sden[:], totd[:], 1e-30)
                        rs = work.tile([P, H], f32, tag="rs")
                        nc.vector.reciprocal(rs[:], sden[:])
                        h1 = work.tile([P, ZW], f32, tag="h1")
                        nc.vector.tensor_tensor(
                            out=h1[:],
                            in0=totz[:],
                            in1=_ap(rs[:], 0, [[1, H], [0, D]]),
                            op=mybir.AluOpType.mult,
                        )
                        nc.sync.dma_start(
                            out=OUT[g * P : (g + 1) * P, :], in_=h1[:]
                        )
                ci += W8
    mybir.codegen_inst_isa_subclasses(nc)
    return nc


# ------------------------------------------------- plan v7 (indirect DMA)
def build_plan_v7(src, dst, n=N, cores=CORES, npc=NPC):
    """Single-phase token streams (int32 indices, no A/B split).

    Tokens grouped by dst group only; idx32 [P, NC] per core; host-built
    one-hot stream OHS [P, NC*P]; tokdst/toksrc for etok.
    """
    key = ("v7", src.tobytes(), dst.tobytes(), n, cores, npc)
    h = hash(key)
    if h in _PLAN_CACHE:
        return _PLAN_CACHE[h]
    import ml_dtypes

    ng = (npc + P - 1) // P
    order = np.argsort(dst, kind="stable")
    ssrc = src[order].astype(np.int64)
    sdst = dst[order].astype(np.int64)
    deg = np.bincount(dst, minlength=n).astype(np.int64)
    starts = np.zeros(n + 1, dtype=np.int64)
    np.cumsum(deg, out=starts[1:])

    eg = [[None] * ng for _ in range(cores)]
    for c in range(cores):
        base = c * npc
        for g in range(ng):
            lo = base + g * P
            hi = min(base + (g + 1) * P, base + npc)
            es_ = ssrc[starts[lo]:starts[hi]]
            ds_ = sdst[starts[lo]:starts[hi]]
            o = np.argsort(es_, kind="stable")
            eg[c][g] = (es_[o], ds_[o] - lo)

    ncX = [max(1, max((len(eg[c][g][0]) + P - 1) // P for c in range(cores)))
           for g in range(ng)]
    NC = sum(ncX)

    cores_arr = []
    for c in range(cores):
        toksrc = np.zeros((P, NC), dtype=np.int64)
        dsl = np.full((P, NC), -1.0, dtype=np.float32)
        col = 0
        for g in range(ng):
            es_, dslot = eg[c][g]
            ne = len(es_)
            j = np.arange(ne)
            toksrc[j % P, col + j // P] = es_
            dsl[j % P, col + j // P] = dslot
            col += ncX[g]
        oh = (dsl[:, :, None] == np.arange(P, dtype=np.float32)[None, None, :])
        oh = np.ascontiguousarray(
            oh.reshape(P, NC * P)).astype(ml_dtypes.bfloat16)
        gcol = np.zeros(NC, dtype=np.int64)
        col = 0
        for g in range(ng):
            gcol[col:col + ncX[g]] = g
            col += ncX[g]
        tokdst = np.where(dsl >= 0, gcol[None, :] * P + dsl, -1.0
                          ).astype(np.int64)
        cores_arr.append({
            "idx32": np.ascontiguousarray(toksrc.astype(np.int32)),
            "oh": oh,
            "tokdst": tokdst,
            "toksrc": toksrc,
        })

    colmap = []
    for g in range(ng):
        for k in range(ncX[g]):
            colmap.append((g, k == 0, k == ncX[g] - 1))

    plan = {"ng": ng, "ncX": ncX, "NC": NC, "colmap": colmap,
            "cores": cores_arr}
    edge_tot = sum(len(eg[c][g][0]) for c in range(cores)
                   for g in range(ng)) / cores
    plan["pad_frac"] = NC * P / max(edge_tot, 1) - 1.0
    _PLAN_CACHE[h] = plan
    return plan


# ------------------------------------------------- edge nc v7 (indirect)
def build_edge_nc_v7(plan, RWE, H, D, n=N, kk=8, nbuf=8):
    """Edge kernel using indirect-DMA gathers (one chunk = kk columns).

    T    [n, RWE]    bf16  row = z (H*D), rest pad
    IDX  [128, NC]   i32   token t=(c*128+p) -> src at [p, c]
    OHS  [128, NC*P] bf16  host-built one-hot stream
    ETOK [128, NC*H] bf16  es[src]+ed[dst] per token (-1e4 pads)
    OUT  [ng*128, H*D] f32 aggregated z (pre-ELU), denominator-normalized
    """
    bass, tile, mybir = _bass_mods()
    from contextlib import ExitStack

    f32 = mybir.dt.float32
    bf16 = mybir.dt.bfloat16
    i32 = mybir.dt.int32

    ZW = H * D
    ng = plan["ng"]
    NC = plan["NC"]
    colmap = plan["colmap"]

    nc = bass.Bass("TRN2")
    T = nc.dram_tensor("tbl", [n, RWE], bf16, kind="ExternalInput")
    IDX = nc.dram_tensor("idx32", [P, NC], i32, kind="ExternalInput")
    OHS = nc.dram_tensor("ohs", [P, NC * P], bf16, kind="ExternalInput")
    ETOK = nc.dram_tensor("etok", [P, NC * H], bf16, kind="ExternalInput")
    OUT = nc.dram_tensor("out", [ng * P, ZW], f32, kind="ExternalOutput")

    chunks = []
    c0 = 0
    while c0 < NC:
        chunks.append((c0, min(kk, NC - c0)))
        c0 += kk

    with _safe_tile_context()(nc) as tc:
        with ExitStack() as ctx:
            const = ctx.enter_context(tc.tile_pool(name="const", bufs=1))
            gath = ctx.enter_context(tc.tile_pool(name="gath", bufs=nbuf))
            ohp = ctx.enter_context(tc.tile_pool(name="ohp", bufs=nbuf))
            work = ctx.enter_context(tc.tile_pool(name="work", bufs=4))
            psum = ctx.enter_context(
                tc.tile_pool(name="psum", bufs=2, space="PSUM")
            )

            idx = const.tile([P, NC], i32)
            nc.sync.dma_start(out=idx[:], in_=IDX[:, :])
            etok = const.tile([P, NC * H], bf16)
            nc.sync.dma_start(out=etok[:], in_=ETOK[:, :])

            for _ in range(nbuf):
                gb = gath.tile([P, kk * RWE], bf16, tag="gt")
                nc.vector.memset(gb[:], 0.0)

            agg_ps = None
            for gi, (c0, cw) in enumerate(chunks):
                gb = gath.tile([P, kk * RWE], bf16, tag="gt")
                nc.gpsimd.indirect_dma_start(
                    out=gb[:, : cw * RWE],
                    out_offset=None,
                    in_=T[:, :],
                    in_offset=bass.IndirectOffsetOnAxis(
                        ap=idx[:, c0: c0 + cw], axis=0
                    ),
                )
                oh_b = ohp.tile([P, kk * P], bf16, tag="oh")
                oh_eng = nc.sync if gi % 2 == 0 else nc.scalar
                oh_eng.dma_start(
                    out=oh_b[:, : cw * P],
                    in_=OHS[:, c0 * P: (c0 + cw) * P],
                )

                # w = exp(lrelu(e)) = max(exp(e), exp(0.2*e)) — both exps
                # on the (idle) ACT engine, one small max on DVE
                EC = cw * H
                e_sl = etok[:, c0 * H: c0 * H + EC]
                ea = work.tile([P, kk * H], f32, tag="ea")
                nc.scalar.activation(
                    out=ea[:, :EC], in_=e_sl,
                    func=mybir.ActivationFunctionType.Exp,
                )
                eb = work.tile([P, kk * H], f32, tag="eb")
                nc.scalar.activation(
                    out=eb[:, :EC], in_=e_sl,
                    func=mybir.ActivationFunctionType.Exp,
                    scale=NEG_SLOPE,
                )
                w_b = work.tile([P, kk * H], bf16, tag="w")
                nc.vector.tensor_tensor(
                    out=w_b[:, :EC], in0=ea[:, :EC], in1=eb[:, :EC],
                    op=mybir.AluOpType.max,
                )
                # m = w*z  (layer1 table is d-major so w broadcast is
                # innermost-contiguous; layer2 H=1 stays d-major trivially)
                m_b = work.tile([P, kk * ZW], bf16, tag="m")
                if H > 1:
                    in1_dims = [[H, cw], [0, D], [1, H]]
                else:
                    in1_dims = [[H, cw], [0, ZW]]
                nc.vector.tensor_tensor(
                    out=m_b[:, : cw * ZW],
                    in0=_ap(gb[:], 0, [[RWE, cw], [1, ZW]]),
                    in1=_ap(w_b[:], 0, in1_dims),
                    op=mybir.AluOpType.mult,
                )
                for j in range(cw):
                    c = c0 + j
                    g, first, last = colmap[c]
                    if first:
                        agg_ps = psum.tile([P, ZW], f32, tag="agg")
                        den_ps = psum.tile([P, H], f32, tag="den")
                    nc.tensor.matmul(
                        out=agg_ps[:],
                        lhsT=oh_b[:, j * P: (j + 1) * P],
                        rhs=m_b[:, j * ZW: (j + 1) * ZW],
                        start=first, stop=last,
                    )
                    nc.tensor.matmul(
                        out=den_ps[:],
                        lhsT=oh_b[:, j * P: (j + 1) * P],
                        rhs=w_b[:, j * H: (j + 1) * H],
                        start=first, stop=last,
                    )
                    if last:
                        sden = work.tile([P, H], f32, tag="sden")
                        nc.vector.tensor_scalar_add(sden[:], den_ps[:], 1e-30)
                        rs = work.tile([P, H], f32, tag="rs")
                        nc.vector.reciprocal(rs[:], sden[:])
                        h1 = work.tile([P, ZW], f32, tag="h1")
                        rs_dims = [[0, D], [1, H]] if H > 1 else [[0, ZW]]
                        nc.vector.tensor_tensor(
                            out=h1[:],
                            in0=agg_ps[:],
                            in1=_ap(rs[:], 0, rs_dims),
                            op=mybir.AluOpType.mult,
                        )
                        nc.sync.dma_start(
                            out=OUT[g * P: (g + 1) * P, :], in_=h1[:]
                        )
    mybir.codegen_inst_isa_subclasses(nc)
    return nc


# ------------------------------------------------- edge nc v8 (swdge+ACT)
def build_edge_nc_v8(plan, RWE, H, D, n=N, npc=NPC, split=SPLIT):
    """v6 pipeline with DVE offload: attention exps on ACT, d-major m_b
    broadcast (layer 1), park copies on ACT, 4 PSUM buffers, OHS loads
    alternating over the sync/scalar HW queues."""
    bass, tile, mybir = _bass_mods()
    from contextlib import ExitStack
    from concourse.library_config import mlp

    f32 = mybir.dt.float32
    bf16 = mybir.dt.bfloat16
    i16 = mybir.dt.int16

    ZW = H * D
    MW = ZW + H
    ng = plan["ng"]
    ncA, ncB = plan["ncA"], plan["ncB"]
    CA, CB = plan["CA"], plan["CB"]
    chunks = plan["chunks"]
    NC = CA + CB

    nc = bass.Bass("TRN2", num_swdge_queues=NQ)
    T = nc.dram_tensor("tbl", [n, RWE], bf16, kind="ExternalInput")
    IDXA = nc.dram_tensor("idxa", [P, CA * 8], i16, kind="ExternalInput")
    IDXB = nc.dram_tensor("idxb", [P, CB * 8], i16, kind="ExternalInput")
    OHS = nc.dram_tensor("ohs", [P, NC * P], bf16, kind="ExternalInput")
    ETOK = nc.dram_tensor("etok", [P, NC * H], bf16, kind="ExternalInput")
    IDENT = nc.dram_tensor("ident", [P, P], bf16, kind="ExternalInput")
    OUT = nc.dram_tensor("out", [ng * P, ZW], f32, kind="ExternalOutput")

    colmap = []
    for phase, ncX in ((0, ncA), (1, ncB)):
        for g in range(ng):
            for k in range(ncX[g]):
                colmap.append((phase, g, k == 0, k == ncX[g] - 1))

    with _safe_tile_context()(nc) as tc:
        with ExitStack() as ctx:
            nc.gpsimd.load_library(mlp)
            const = ctx.enter_context(tc.tile_pool(name="const", bufs=1))
            gath = ctx.enter_context(tc.tile_pool(name="gath", bufs=8))
            ohp = ctx.enter_context(tc.tile_pool(name="ohp", bufs=8))
            work = ctx.enter_context(tc.tile_pool(name="work", bufs=4))
            psum = ctx.enter_context(
                tc.tile_pool(name="psum", bufs=6, space="PSUM")
            )

            idxa = const.tile([P, CA * 8], i16)
            nc.sync.dma_start(out=idxa[:], in_=IDXA[:, :])
            idxb = const.tile([P, CB * 8], i16)
            nc.sync.dma_start(out=idxb[:], in_=IDXB[:, :])
            etok = const.tile([P, NC * H], bf16)
            nc.sync.dma_start(out=etok[:], in_=ETOK[:, :])
            ident = const.tile([P, P], bf16)
            nc.sync.dma_start(out=ident[:], in_=IDENT[:, :])
            park = const.tile([P, ng * MW], bf16)

            for _ in range(8):
                gb = gath.tile([P, GCOLS * RWE], bf16, tag="gt")
                nc.vector.memset(gb[:], 0.0)

            ni_reg = nc.gpsimd.to_reg(GTOK)

            agg_ps = None
            ci = 0
            for gi, (phase, t0, nt) in enumerate(chunks):
                W8 = nt // P
                idxt = idxa if phase == 0 else idxb
                gb = gath.tile([P, GCOLS * RWE], bf16, tag="gt")
                src_t = T[:split, :] if phase == 0 else T[split:, :]
                reg = ni_reg if nt == GTOK else nt
                nc.gpsimd.dma_gather(
                    _ap(gb[:], 0, [[RWE, W8], [1, RWE]]),
                    src_t,
                    idxt[:, t0 // 16: t0 // 16 + nt // 16],
                    nt, reg, RWE,
                    queue_num=gi % NQ,
                )
                oh_b = ohp.tile([P, GCOLS * P], bf16, tag="oh")
                oh_eng = nc.sync if gi % 2 == 0 else nc.scalar
                oh_eng.dma_start(
                    out=oh_b[:, : W8 * P],
                    in_=OHS[:, ci * P: (ci + W8) * P],
                )

                # w = exp(lrelu(e)) = max(exp(e), exp(0.2 e)); exps on ACT
                EC = W8 * H
                e_sl = etok[:, ci * H: ci * H + EC]
                ea = work.tile([P, GCOLS * H], f32, tag="ea")
                nc.scalar.activation(
                    out=ea[:, :EC], in_=e_sl,
                    func=mybir.ActivationFunctionType.Exp,
                )
                eb = work.tile([P, GCOLS * H], f32, tag="eb")
                nc.scalar.activation(
                    out=eb[:, :EC], in_=e_sl,
                    func=mybir.ActivationFunctionType.Exp,
                    scale=NEG_SLOPE,
                )
                w_b = work.tile([P, GCOLS * H], bf16, tag="w")
                nc.vector.tensor_tensor(
                    out=w_b[:, :EC], in0=ea[:, :EC], in1=eb[:, :EC],
                    op=mybir.AluOpType.max,
                )
                # m = [w*z | w] (z d-major for H>1: w broadcast contiguous)
                m_b = work.tile([P, GCOLS * MW], bf16, tag="m")
                if H > 1:
                    in1_dims = [[H, W8], [0, D], [1, H]]
                else:
                    in1_dims = [[H, W8], [0, ZW]]
                nc.vector.tensor_tensor(
                    out=_ap(m_b[:], 0, [[MW, W8], [1, ZW]]),
                    in0=_ap(gb[:], 0, [[RWE, W8], [1, ZW]]),
                    in1=_ap(w_b[:], 0, in1_dims),
                    op=mybir.AluOpType.mult,
                )
                nc.vector.tensor_copy(
                    out=_ap(m_b[:], ZW, [[MW, W8], [1, H]]),
                    in_=_ap(w_b[:], 0, [[H, W8], [1, H]]),
                )
                for j in range(W8):
                    c = ci + j
                    phase_c, g, first, last = colmap[c]
                    if first:
                        agg_ps = psum.tile([P, MW], f32, tag="agg")
                        if phase_c == 1:
                            # reinject parked phase-A partials via PE
                            nc.tensor.matmul(
                                out=agg_ps[:],
                                lhsT=ident[:],
                                rhs=_ap(park[:], g * MW, [[1, MW]]),
                                start=True, stop=False,
                            )
                    nc.tensor.matmul(
                        out=agg_ps[:],
                        lhsT=oh_b[:, j * P: (j + 1) * P],
                        rhs=m_b[:, j * MW: (j + 1) * MW],
                        start=(first and phase_c == 0), stop=last,
                    )
                    if last and phase_c == 0:
                        nc.scalar.activation(
                            out=_ap(park[:], g * MW, [[1, MW]]),
                            in_=agg_ps[:],
                            func=mybir.ActivationFunctionType.Copy,
                        )
                    if last and phase_c == 1:
                        sden = work.tile([P, H], f32, tag="sden")
                        nc.scalar.activation(
                            out=sden[:], in_=agg_ps[:, ZW:MW],
                            func=mybir.ActivationFunctionType.Copy,
                            bias=1e-30,
                        )
                        rs = work.tile([P, H], f32, tag="rs")
                        nc.vector.reciprocal(rs[:], sden[:])
                        h1 = work.tile([P, ZW], f32, tag="h1")
                        rs_dims = [[0, D], [1, H]] if H > 1 else [[0, ZW]]
                        nc.vector.tensor_tensor(
                            out=h1[:], in0=agg_ps[:, :ZW],
                            in1=_ap(rs[:], 0, rs_dims),
                            op=mybir.AluOpType.mult,
                        )
                        nc.sync.dma_start(
                            out=OUT[g * P: (g + 1) * P, :], in_=h1[:]
                        )
                ci += W8
    mybir.codegen_inst_isa_subclasses(nc)
    return nc


# ---------------------------------------------------------------- dense nc
def build_dense_nc(elu_in=False):
    """out[tile] = elu?(xT)[:, tile].T @ Waug -> [NT*P, DENSE_W] (fp32)."""
    bass, tile, mybir = _bass_mods()
    from contextlib import ExitStack

    f32 = mybir.dt.float32
    nc = bass.Bass("TRN2")
    xT = nc.dram_tensor("xt", [P, NT * P], f32, kind="ExternalInput")
    W = nc.dram_tensor("waug", [P, DENSE_W], f32, kind="ExternalInput")
    OUTD = nc.dram_tensor("outd", [NT * P, DENSE_W], f32, kind="ExternalOutput")

    with _safe_tile_context()(nc) as tc:
        with ExitStack() as ctx:
            const = ctx.enter_context(tc.tile_pool(name="const", bufs=1))
            work = ctx.enter_context(tc.tile_pool(name="work", bufs=3))
            psum = ctx.enter_context(tc.tile_pool(name="psum", bufs=4, space="PSUM"))

            wsb = const.tile([P, DENSE_W], f32)
            nc.sync.dma_start(out=wsb[:], in_=W[:, :])
            xsb = const.tile([P, NT * P], f32)
            nc.sync.dma_start(out=xsb[:], in_=xT[:, :])
            if elu_in:
                pos = const.tile([P, NT * P], f32)
                nc.vector.tensor_scalar_max(pos[:], xsb[:], 0.0)
                ngx = const.tile([P, NT * P], f32)
                nc.vector.tensor_scalar_min(ngx[:], xsb[:], 0.0)
                ex = const.tile([P, NT * P], f32)
                nc.scalar.activation(
                    out=ex[:], in_=ngx[:],
                    func=mybir.ActivationFunctionType.Exp,
                )
                nc.vector.tensor_tensor(
                    out=xsb[:], in0=pos[:], in1=ex[:], op=mybir.AluOpType.add
                )
                nc.vector.tensor_scalar_add(xsb[:], xsb[:], -1.0)

            for t in range(NT):
                ps = psum.tile([P, DENSE_W], f32, tag="ps")
                nc.tensor.matmul(
                    out=ps[:],
                    lhsT=xsb[:, t * P : (t + 1) * P],
                    rhs=wsb[:],
                    start=True,
                    stop=True,
                )
                st = work.tile([P, DENSE_W], f32, tag="st")
                nc.vector.tensor_copy(out=st[:], in_=ps[:])
                nc.sync.dma_start(out=OUTD[t * P : (t + 1) * P, :], in_=st[:])
    return nc


# ---------------------------------------------------------------- run layer
def _run_spmd(nc, in_maps, collect, label):
    from concourse.bass_utils import run_bass_kernel_spmd

    trace = bool(int(os.environ.get("GAT_TRACE", "0")))
    res = run_bass_kernel_spmd(
        nc, in_maps, core_ids=list(range(CORES)), trace=trace
    )
    if collect is not None:
        collect.append((label, getattr(res, "exec_time_ns", None)))
    return res.results


def _dense_phase(x, Waug, collect, label, elu_in=False):
    xT = np.ascontiguousarray(x.T.astype(np.float32))
    xT_pad = np.zeros((P, NT * P), dtype=np.float32)
    in_maps = []
    for c in range(CORES):
        xc = np.array(xT_pad)
        xc[:, :NPC] = xT[:, c * NPC : (c + 1) * NPC]
        in_maps.append({"xt": xc, "waug": Waug})
    outs = _run_spmd(build_dense_nc(elu_in), in_maps, collect, label)
    return np.concatenate([o["outd"][:NPC] for o in outs], axis=0)


def _edge_phase_v4(dense_full, plan, RWE, H, D, elu, collect, label):
    import ml_dtypes

    ZW = H * D
    ng = plan["ng"]
    ver0 = os.environ.get("GAT_V4_VER", "v6")
    tbl = np.zeros((N, RWE), dtype=ml_dtypes.bfloat16)
    tbl[:, :ZW] = dense_full[:, :ZW].astype(ml_dtypes.bfloat16)
    if ver0 != "v6":
        tbl[:, ZW : ZW + H] = dense_full[:, ZW : ZW + H].astype(
            ml_dtypes.bfloat16)
    iota = np.broadcast_to(
        np.arange(P, dtype=np.float32)[None, :], (P, P)
    ).astype(ml_dtypes.bfloat16)
    ident = np.eye(P, dtype=np.float32).astype(ml_dtypes.bfloat16)
    ver = os.environ.get("GAT_V4_VER", "v6")
    in_maps = []
    for c in range(CORES):
        pc = plan["cores"][c]
        edc = np.zeros((ng * P, H), dtype=ml_dtypes.bfloat16)
        edc[:NPC] = dense_full[
            c * NPC : (c + 1) * NPC, ZW + H : ZW + 2 * H
        ].astype(ml_dtypes.bfloat16)
        if ver == "v6":
            esl = dense_full[:, ZW : ZW + H]                  # es per node
            edl = np.zeros((ng * P, H), dtype=np.float32)
            edl[:NPC] = dense_full[
                c * NPC : (c + 1) * NPC, ZW + H : ZW + 2 * H
            ]
            td = pc["tokdst"]
            tsrc = pc["toksrc"]
            etok = np.where(
                (td >= 0)[:, :, None],
                esl[tsrc] + edl[np.maximum(td, 0)],
                -1.0e4,
            )
            etok = np.ascontiguousarray(
                etok.reshape(P, -1)).astype(ml_dtypes.bfloat16)
            in_maps.append(
                {
                    "tbl": tbl,
                    "idxa": pc["idxA"],
                    "idxb": pc["idxB"],
                    "ohs": pc["oh"],
                    "etok": etok,
                }
            )
        elif ver == "v5":
            edl = np.zeros((ng * P, H), dtype=np.float32)
            edl[:NPC] = dense_full[
                c * NPC : (c + 1) * NPC, ZW + H : ZW + 2 * H
            ]
            td = pc["tokdst"]
            edt = np.where(
                (td >= 0)[:, :, None], edl[np.maximum(td, 0)], 0.0
            )
            edt = np.ascontiguousarray(
                edt.reshape(P, -1)).astype(ml_dtypes.bfloat16)
            in_maps.append(
                {
                    "tbl": tbl,
                    "idxa": pc["idxA"],
                    "idxb": pc["idxB"],
                    "ohs": pc["oh"],
                    "edt": edt,
                }
            )
        else:
            in_maps.append(
                {
                    "tbl": tbl,
                    "idxa": pc["idxA"],
                    "idxb": pc["idxB"],
                    "dsl": pc["dsl"],
                    "edc": edc,
                    "iota": np.ascontiguousarray(iota),
                    "ident": ident,
                }
            )
    ver = os.environ.get("GAT_V4_VER", "v6")
    if ver == "v4":
        nc = build_edge_nc_v4(plan, RWE, H, D, elu)
    elif ver == "v4b":
        nc = build_edge_nc_v4b(plan, RWE, H, D, elu)
    elif ver == "v5":
        nc = build_edge_nc_v5(plan, RWE, H, D, elu)
    else:
        nc = build_edge_nc_v6(plan, RWE, H, D, elu)
    outs = _run_spmd(nc, in_maps, collect, label)
    return np.concatenate([o["out"][:NPC] for o in outs], axis=0)


# ------------------------------------------------- edge phase v7 host glue
def _edge_phase_v7(dense_full, plan, RWE, H, D, collect, label):
    import ml_dtypes

    ZW = H * D
    ng = plan["ng"]
    tbl = np.zeros((N, RWE), dtype=ml_dtypes.bfloat16)
    tbl[:, :ZW] = dense_full[:, :ZW].astype(ml_dtypes.bfloat16)
    esl = dense_full[:, ZW:ZW + H]
    in_maps = []
    for c in range(CORES):
        pc = plan["cores"][c]
        edl = np.zeros((ng * P, H), dtype=np.float32)
        edl[:NPC] = dense_full[c * NPC:(c + 1) * NPC, ZW + H:ZW + 2 * H]
        td = pc["tokdst"]
        tsrc = pc["toksrc"]
        etok = np.where((td >= 0)[:, :, None],
                        esl[tsrc] + edl[np.maximum(td, 0)], -1.0e4)
        etok = np.ascontiguousarray(
            etok.reshape(P, -1)).astype(ml_dtypes.bfloat16)
        in_maps.append({"tbl": tbl, "idx32": pc["idx32"], "ohs": pc["oh"],
                        "etok": etok})
    nc = build_edge_nc_v7(plan, RWE, H, D)
    outs = _run_spmd(nc, in_maps, collect, label)
    return np.concatenate([o["out"][:NPC] for o in outs], axis=0)


def _edge_phase_v8(dense_full, plan, RWE, H, D, collect, label):
    import ml_dtypes

    ZW = H * D
    ng = plan["ng"]
    tbl = np.zeros((N, RWE), dtype=ml_dtypes.bfloat16)
    tbl[:, :ZW] = dense_full[:, :ZW].astype(ml_dtypes.bfloat16)
    esl = dense_full[:, ZW:ZW + H]
    in_maps = []
    for c in range(CORES):
        pc = plan["cores"][c]
        edl = np.zeros((ng * P, H), dtype=np.float32)
        edl[:NPC] = dense_full[c * NPC:(c + 1) * NPC, ZW + H:ZW + 2 * H]
        td = pc["tokdst"]
        tsrc = pc["toksrc"]
        etok = np.where((td >= 0)[:, :, None],
                        esl[tsrc] + edl[np.maximum(td, 0)], -1.0e4)
        etok = np.ascontiguousarray(
            etok.reshape(P, -1)).astype(ml_dtypes.bfloat16)
        in_maps.append({"tbl": tbl, "idxa": pc["idxA"], "idxb": pc["idxB"],
                        "ohs": pc["oh"], "etok": etok,
                        "ident": np.eye(P, dtype=np.float32).astype(
                            ml_dtypes.bfloat16)})
    nc = build_edge_nc_v8(plan, RWE, H, D)
    outs = _run_spmd(nc, in_maps, collect, label)
    return np.concatenate([o["out"][:NPC] for o in outs], axis=0)


def _kernel_v8(h, W1a, W2a, src, dst, _collect):
    perm = np.array([[hh * HID + dd for hh in range(HEADS)]
                     for dd in range(HID)]).reshape(-1)
    W1a_p = np.array(W1a)
    W1a_p[:, :HEADS * HID] = W1a[:, perm]
    W2a_p = np.array(W2a)
    W2a_p[:HEADS * HID, :] = W2a[perm, :]

    plan = build_plan_v4(src, dst)
    d1 = _dense_phase(h, W1a_p, _collect, "dense1")
    h1 = _edge_phase_v8(d1, plan, RWE=128, H=HEADS, D=HID,
                        collect=_collect, label="edge1")
    d2 = _dense_phase(h1, W2a_p, _collect, "dense2", elu_in=True)
    out = _edge_phase_v8(d2, plan, RWE=128, H=1, D=OUT_DIM,
                         collect=_collect, label="edge2")
    return out.astype(np.float32)


def _kernel_v7(h, W1a, W2a, src, dst, _collect):
    # d-major permutation of layer-1 hidden features: new col d*H+h_ =
    # old col h_*HID+d.  Applied to W1a's z columns and W2a's rows, so
    # the on-device layouts stay consistent and the final output is
    # unpermuted.
    perm = np.array([[hh * HID + dd for hh in range(HEADS)]
                     for dd in range(HID)]).reshape(-1)
    W1a_p = np.array(W1a)
    W1a_p[:, :HEADS * HID] = W1a[:, perm]
    W2a_p = np.array(W2a)
    W2a_p[:HEADS * HID, :] = W2a[perm, :]

    plan = build_plan_v7(src, dst)
    d1 = _dense_phase(h, W1a_p, _collect, "dense1")
    h1 = _edge_phase_v7(d1, plan, RWE=128, H=HEADS, D=HID,
                        collect=_collect, label="edge1")
    d2 = _dense_phase(h1, W2a_p, _collect, "dense2", elu_in=True)
    out = _edge_phase_v7(d2, plan, RWE=128, H=1, D=OUT_DIM,
                         collect=_collect, label="edge2")
    return out.astype(np.float32)


# ---------------------------------------------------------------- kernel
def kernel(h, W1, a1_src, a1_dst, W2, a2_src, a2_dst, src, dst, _collect=None):
    h = np.asarray(h, dtype=np.float32)
    W1 = np.asarray(W1, dtype=np.float32)
    W2 = np.asarray(W2, dtype=np.float32)
    a1_src = np.asarray(a1_src, dtype=np.float32)
    a1_dst = np.asarray(a1_dst, dtype=np.float32)
    a2_src = np.asarray(a2_src, dtype=np.float32)
    a2_dst = np.asarray(a2_dst, dtype=np.float32)
    src = np.asarray(src)
    dst = np.asarray(dst)

    W1a = fuse_weights(W1, a1_src, a1_dst, HEADS, HID)
    W2a = fuse_weights(W2, a2_src, a2_dst, 1, OUT_DIM)

    ver = os.environ.get("GAT_V4_VER", "v8")
    if ver == "v8":
        return _kernel_v8(h, W1a, W2a, src, dst, _collect)
    if ver == "v7":
        return _kernel_v7(h, W1a, W2a, src, dst, _collect)
    plan = build_plan_v4(src, dst)
    rw1 = 128 if ver == "v6" else 256
    rw2 = 128
    elu1 = ver != "v6"
    d1 = _dense_phase(h, W1a, _collect, "dense1")
    h1 = _edge_phase_v4(d1, plan, RWE=rw1, H=HEADS, D=HID, elu=elu1,
                        collect=_collect, label="edge1")
    d2 = _dense_phase(h1, W2a, _collect, "dense2",
                      elu_in=(ver == "v6"))
    out = _edge_phase_v4(d2, plan, RWE=rw2, H=1, D=OUT_DIM, elu=False,
                         collect=_collect, label="edge2")
    return out.astype(np.float32)



# revision 13
# speedup vs baseline: 1.0699x; 1.0699x over previous
"""Two-layer GAT on 8 TRN2 cores — v4: dma_gather edge phase.

Edge phase redesign vs v3 (indirect_dma_start, 1.4us per 128 rows):
  * Per core, dst nodes in NG groups of 128 (one PSUM row each).  Each
    group's edges are split by src < SPLIT (int16 index limit of
    dma_gather) into A/B runs, sorted by src, padded to 128-token
    columns.  Two token streams (A then B) are fetched with ~1024-token
    dma_gather instructions round-robined over 4 SWDGE queues
    (~3.1 ns/token measured vs ~11 ns/token for indirect DMA).
  * Gathered row = [z | es] bf16 of the edge's src node.  Token t lands
    at partition t%128, free slot t//128.
  * Per column (128 tokens): one-hot oh[t,d] = (dstslot[t]==d) built on
    DVE; ohT via PE transpose (matmul with identity); per-token ed via
    matmul(lhsT=ohT, rhs=ed_group); w = exp(leakyrelu(es+ed)) on
    DVE/ACT; messages m = [w*z | w] on DVE; aggregation via
    matmul(lhsT=oh, rhs=m) accumulated in a per-group PSUM tile.
  * Phase A results are parked in SBUF; phase B accumulates its own
    PSUM tile; epilogue adds both, divides by the summed weights,
    applies ELU (layer 1) and writes 128 output rows sequentially (no
    indirect scatter).
  * Pad tokens point at table row 0 and carry dstslot=-1, so their
    one-hot column is zero and they contribute nothing.
"""

import os
import sys

import numpy as np

for _p in ("/opt/trn_rl_repo", "/root/.axon_site/_ro/trn_rl_repo"):
    if os.path.isdir(_p) and _p not in sys.path:
        sys.path.insert(0, _p)

# ---------------------------------------------------------------- constants
N = 50000
E = 800000
IN_DIM = 128
HID = 16
HEADS = 8
OUT_DIM = 32
NEG_SLOPE = 0.2

CORES = 8
NPC = N // CORES          # nodes per core
P = 128
SPLIT = 32768             # table-A rows (int16 index limit)
NG = (NPC + P - 1) // P   # dst groups per core (49)
GTOK = 1024               # tokens per dma_gather (ring cap ~1.5k)
GCOLS = GTOK // P         # 8 columns per gather
NQ = 4                    # SWDGE queues
DENSE_W = 144
NT = (NPC + P - 1) // P

_PLAN_CACHE = {}


def _bass_mods():
    import concourse.bass as bass
    import concourse.tile as tile
    from concourse import mybir

    return bass, tile, mybir


_SAFE_TC = None


def _safe_tile_context():
    """TileContext whose kernel-tail drain never carries more than 2 sem
    waits per instruction (this container's walrus rejects >2 sync-wait
    commands on the SP CTRL drain); excess waits are moved onto preceding
    SP nops."""
    global _SAFE_TC
    if _SAFE_TC is not None:
        return _SAFE_TC
    import concourse.tile as tile
    from concourse import mybir
    from concourse.vector_clock import ScopedClock

    class TileContextSafe(tile.TileContext):
        def _add_instruction(self, inst):
            si = inst.sync_info
            if (
                si is not None
                and si.on_wait
                and len(si.on_wait) > 1
                and inst.engine != mybir.EngineType.Unassigned
            ):
                waits = list(si.on_wait)
                si.on_wait = waits[-1:]
                for w in waits[:-1]:
                    nop = mybir.InstNoOp(
                        name=self.nc.get_next_instruction_name(), ins=[], outs=[]
                    )
                    nop.engine = inst.engine
                    nop.sync_info = mybir.SyncInfo(on_wait=[w], on_update=[])
                    super()._add_instruction(nop)
            super()._add_instruction(inst)

        def _drain_and_barrier(self, tick_clock, wait_clock):
            nc = self.nc
            nops = [nc.sync.nop(nofuse=True) for _ in range(28)]
            drain_inst = nc.sync.drain()
            wait_clock.add_sem_waits(
                drain_inst.ins, ScopedClock({None: tick_clock.global_clock})
            )
            si = drain_inst.ins.sync_info
            waits = list(si.on_wait) if si is not None and si.on_wait else []
            if len(waits) > 1:
                si.on_wait = waits[:1]
                rest = waits[1:]
                assert len(rest) <= len(nops), "raise nop count"
                for k, w in enumerate(rest):
                    nops[k].ins.sync_info = mybir.SyncInfo(
                        on_wait=[w], on_update=[]
                    )

            nc.all_engine_barrier()
            assert self.sems is not None
            popped = nc._tile_sem_poison_stack.pop()
            assert popped is self._sem_poison
            nc.clear_and_free_semaphores(list(self.sems.allocated().values()))
            nc.all_engine_barrier()

    _SAFE_TC = TileContextSafe
    return _SAFE_TC


def _ap(tile_ap, col_off, dims):
    import concourse.bass as bass

    part = list(tile_ap.ap[0])
    return bass.AP(
        tile_ap.tensor,
        tile_ap.offset + col_off,
        [part] + [list(d) for d in dims],
    )


# ---------------------------------------------------------------- host prep
def fuse_weights(W, a_src, a_dst, H, D):
    """W:[K, H*D] -> [K, DENSE_W] = [W | Wes | Wed] (zero padded)."""
    K = W.shape[0]
    Wr = W.reshape(K, H, D)
    wes = np.einsum("khd,hd->kh", Wr, a_src)
    wed = np.einsum("khd,hd->kh", Wr, a_dst)
    out = np.zeros((K, DENSE_W), dtype=np.float32)
    out[:, : H * D] = W
    out[:, H * D : H * D + H] = wes
    out[:, H * D + H : H * D + 2 * H] = wed
    return out


def pack_idx16(tok):
    """[T] int -> [128, T//16] int16; token t at [t%16, t//16], replicated
    across the 8 groups of 16 partitions."""
    T = len(tok)
    assert T % 16 == 0
    a = np.asarray(tok, dtype=np.int16).reshape(T // 16, 16).T  # [16, T/16]
    return np.tile(a, (8, 1))


def build_plan_v4(src, dst, n=N, cores=CORES, npc=NPC, split=SPLIT):
    """Token streams for the v4 edge kernel (layer-independent).

    Static (shared across cores): ncA/ncB columns per group, gather
    chunk list.  Per core: int16 index streams, dstslot array.
    """
    key = ("v4", src.tobytes(), dst.tobytes(), n, cores, npc, split)
    h = hash(key)
    if h in _PLAN_CACHE:
        return _PLAN_CACHE[h]

    ng = (npc + P - 1) // P
    order = np.argsort(dst, kind="stable")
    ssrc = src[order].astype(np.int64)
    sdst = dst[order].astype(np.int64)
    core_of = sdst // npc
    deg = np.bincount(dst, minlength=n).astype(np.int64)
    starts = np.zeros(n + 1, dtype=np.int64)
    np.cumsum(deg, out=starts[1:])

    # per (core, group): A/B edge lists sorted by src
    eA = [[None] * ng for _ in range(cores)]
    eB = [[None] * ng for _ in range(cores)]
    for c in range(cores):
        base = c * npc
        for g in range(ng):
            lo = base + g * P
            hi = min(base + (g + 1) * P, base + npc)
            es_ = ssrc[starts[lo] : starts[hi]]
            ds_ = sdst[starts[lo] : starts[hi]]
            o = np.argsort(es_, kind="stable")
            es_, ds_ = es_[o], ds_[o]
            half = np.searchsorted(es_, split)
            eA[c][g] = (es_[:half], ds_[:half] - lo)
            eB[c][g] = (es_[half:] - split, ds_[half:] - lo)

    ncA = [
        max(1, max((len(eA[c][g][0]) + P - 1) // P for c in range(cores)))
        for g in range(ng)
    ]
    ncB = [
        max(1, max((len(eB[c][g][0]) + P - 1) // P for c in range(cores)))
        for g in range(ng)
    ]
    CA, CB = sum(ncA), sum(ncB)

    cores_arr = []
    for c in range(cores):
        tokA = np.zeros(CA * P, dtype=np.int16)
        tokB = np.zeros(CB * P, dtype=np.int16)
        import ml_dtypes
        dsl = np.full((P, CA + CB), -1.0, dtype=ml_dtypes.bfloat16)
        for phase, (toks, ncX, eX, coff) in enumerate(
            (
                (tokA, ncA, eA, 0),
                (tokB, ncB, eB, CA),
            )
        ):
            t0 = 0
            col = coff
            for g in range(ng):
                es_, dslot = eX[c][g]
                ne = len(es_)
                toks[t0 : t0 + ne] = es_.astype(np.int16)
                j = np.arange(ne)
                dsl[j % P, col + j // P] = dslot.astype(ml_dtypes.bfloat16)
                t0 += ncX[g] * P
                col += ncX[g]
        dslf = dsl.astype(np.float32)
        oh = (dslf[:, :, None] == np.arange(P, dtype=np.float32)[None, None, :])
        oh = np.ascontiguousarray(
            oh.reshape(P, (CA + CB) * P)).astype(ml_dtypes.bfloat16)
        gcol = np.zeros(CA + CB, dtype=np.int64)   # group of each column
        col = 0
        for ph, ncX in ((0, ncA), (1, ncB)):
            for g in range(ng):
                gcol[col : col + ncX[g]] = g
                col += ncX[g]
        tokdst = np.where(
            dslf >= 0, gcol[None, :] * P + dslf, -1.0
        ).astype(np.int64)
        tsA = np.zeros(CA * P, dtype=np.int64)
        tsB = np.zeros(CB * P, dtype=np.int64)
        for toks2, ncX, eX, off2 in (
            (tsA, ncA, eA, 0), (tsB, ncB, eB, split)
        ):
            t0b = 0
            for g in range(ng):
                es2, _ = eX[c][g]
                toks2[t0b : t0b + len(es2)] = es2 + off2
                t0b += ncX[g] * P
        allt = np.concatenate([tsA, tsB])
        toksrc = np.zeros((P, CA + CB), dtype=np.int64)
        tt = np.arange(len(allt))
        toksrc[tt % P, tt // P] = allt
        cores_arr.append(
            {
                "idxA": pack_idx16(tokA),
                "idxB": pack_idx16(tokB),
                "dsl": dsl,
                "oh": oh,
                "tokdst": tokdst,
                "toksrc": toksrc,
            }
        )

    # gather chunks: (phase, token_start_in_stream, ntok)
    chunks = []
    for phase, CX in ((0, CA), (1, CB)):
        t = 0
        while t < CX * P:
            nt = min(GTOK, CX * P - t)
            chunks.append((phase, t, nt))
            t += nt

    plan = {
        "ng": ng,
        "ncA": ncA,
        "ncB": ncB,
        "CA": CA,
        "CB": CB,
        "chunks": chunks,
        "cores": cores_arr,
    }
    tok_tot = (CA + CB) * P
    edge_tot = sum(len(eA[c][g][0]) + len(eB[c][g][0])
                   for c in range(cores) for g in range(ng)) / cores
    plan["pad_frac"] = tok_tot / max(edge_tot, 1) - 1.0
    _PLAN_CACHE[h] = plan
    return plan


# ---------------------------------------------------------------- edge nc
def build_edge_nc_v4(plan, RWE, H, D, elu, n=N, npc=NPC, split=SPLIT):
    """Edge kernel for one GAT layer (one program, SPMD over cores).

    T    [n, RWE]  bf16  row = [z (H*D) | es (H) | pad]
    IDXA [128, CA*8] i16; IDXB [128, CB*8] i16
    DSL  [128, CA+CB] f32   dst slot per token (-1 = pad)
    EDC  [ng*128, H] bf16   ed rows of this core's nodes
    IOTA [128, 128] bf16    iota[p, j] = j
    IDENT[128, 128] bf16    identity
    OUT  [ng*128, H*D] f32
    """
    bass, tile, mybir = _bass_mods()
    from contextlib import ExitStack
    from concourse.library_config import mlp

    f32 = mybir.dt.float32
    bf16 = mybir.dt.bfloat16
    i16 = mybir.dt.int16

    ZW = H * D
    MW = ZW + H
    ng = plan["ng"]
    ncA, ncB = plan["ncA"], plan["ncB"]
    CA, CB = plan["CA"], plan["CB"]
    chunks = plan["chunks"]

    nc = bass.Bass("TRN2", num_swdge_queues=NQ)
    T = nc.dram_tensor("tbl", [n, RWE], bf16, kind="ExternalInput")
    IDXA = nc.dram_tensor("idxa", [P, CA * 8], i16, kind="ExternalInput")
    IDXB = nc.dram_tensor("idxb", [P, CB * 8], i16, kind="ExternalInput")
    DSL = nc.dram_tensor("dsl", [P, CA + CB], bf16, kind="ExternalInput")
    EDC = nc.dram_tensor("edc", [ng * P, H], bf16, kind="ExternalInput")
    IOTA = nc.dram_tensor("iota", [P, P], bf16, kind="ExternalInput")
    IDENT = nc.dram_tensor("ident", [P, P], bf16, kind="ExternalInput")
    OUT = nc.dram_tensor("out", [ng * P, ZW], f32, kind="ExternalOutput")

    # column -> (phase, group, first?, last?) map
    colmap = []
    for phase, ncX in ((0, ncA), (1, ncB)):
        for g in range(ng):
            for k in range(ncX[g]):
                colmap.append((phase, g, k == 0, k == ncX[g] - 1))

    with _safe_tile_context()(nc) as tc:
        with ExitStack() as ctx:
            nc.gpsimd.load_library(mlp)
            const = ctx.enter_context(tc.tile_pool(name="const", bufs=1))
            gath = ctx.enter_context(tc.tile_pool(name="gath", bufs=6))
            meta = ctx.enter_context(tc.tile_pool(name="meta", bufs=2))
            work = ctx.enter_context(tc.tile_pool(name="work", bufs=3))
            psum = ctx.enter_context(
                tc.tile_pool(name="psum", bufs=2, space="PSUM")
            )
            psT = ctx.enter_context(
                tc.tile_pool(name="psT", bufs=3, space="PSUM")
            )

            iota = const.tile([P, P], bf16)
            nc.sync.dma_start(out=iota[:], in_=IOTA[:, :])
            ident = const.tile([P, P], bf16)
            nc.sync.dma_start(out=ident[:], in_=IDENT[:, :])
            idxa = const.tile([P, CA * 8], i16)
            nc.sync.dma_start(out=idxa[:], in_=IDXA[:, :])
            idxb = const.tile([P, CB * 8], i16)
            nc.sync.dma_start(out=idxb[:], in_=IDXB[:, :])
            dsl = const.tile([P, CA + CB], bf16)
            nc.sync.dma_start(out=dsl[:], in_=DSL[:, :])
            park = const.tile([P, ng * MW], f32)
            edg_all = const.tile([P, ng * H], bf16)
            for g in range(ng):
                nc.sync.dma_start(
                    out=edg_all[:, g * H : (g + 1) * H],
                    in_=EDC[g * P : (g + 1) * P, :],
                )

            # pre-zero the rotating gather buffers (stale SBUF may be NaN;
            # pad-token garbage must stay finite)
            gbufs = []
            for _ in range(6):
                gb = gath.tile([P, GCOLS * RWE], bf16, tag="gt")
                nc.vector.memset(gb[:], 0.0)
                gbufs.append(gb)

            ni_reg = nc.gpsimd.to_reg(GTOK)

            agg_ps = None   # current group's PSUM agg tile
            ci = 0          # global column index
            for gi, (phase, t0, nt) in enumerate(chunks):
              idxt = idxa if phase == 0 else idxb
              gb = gath.tile([P, GCOLS * RWE], bf16, tag="gt")
              src_t = T[:split, :] if phase == 0 else T[split:, :]
              reg = ni_reg if nt == GTOK else nt
              nc.gpsimd.dma_gather(
                  _ap(gb[:], 0, [[RWE, (nt + P - 1) // P], [1, RWE]]),
                  src_t,
                  idxt[:, t0 // 16 : t0 // 16 + (nt + 15) // 16],
                  nt,
                  reg,
                  RWE,
                  queue_num=gi % NQ,
              )
              for slot in range(nt // P):
                c = ci
                ci += 1
                phase_c, g, first, last = colmap[c]
                assert phase_c == phase
                gbase = slot * RWE

                # one-hot oh[t, d]
                oh = work.tile([P, P], bf16, tag="oh")
                nc.vector.tensor_tensor(
                    out=oh[:],
                    in0=_ap(dsl[:], c, [[0, P]]),
                    in1=iota[:],
                    op=mybir.AluOpType.is_equal,
                )
                # ohT via PE transpose
                pst = psT.tile([P, P], f32, tag="ohT")
                nc.tensor.matmul(
                    out=pst[:], lhsT=oh[:], rhs=ident[:], start=True, stop=True
                )
                ohT = work.tile([P, P], bf16, tag="ohTs")
                nc.scalar.activation(
                    out=ohT[:], in_=pst[:],
                    func=mybir.ActivationFunctionType.Copy,
                )
                # per-token ed
                psed = psT.tile([P, H], f32, tag="ed")
                nc.tensor.matmul(
                    out=psed[:], lhsT=ohT[:],
                    rhs=edg_all[:, g * H : (g + 1) * H],
                    start=True, stop=True,
                )
                # w = exp(leakyrelu(es + ed))
                es_f = work.tile([P, H], f32, tag="esf")
                nc.vector.tensor_copy(
                    out=es_f[:], in_=_ap(gb[:], gbase + ZW, [[1, H]])
                )
                e_t = work.tile([P, H], f32, tag="e")
                nc.vector.tensor_tensor(
                    out=e_t[:],
                    in0=es_f[:],
                    in1=psed[:],
                    op=mybir.AluOpType.add,
                )
                t2 = work.tile([P, H], f32, tag="t2")
                nc.vector.tensor_scalar_mul(t2[:], e_t[:], NEG_SLOPE)
                t3 = work.tile([P, H], f32, tag="t3")
                nc.vector.tensor_tensor(
                    out=t3[:], in0=e_t[:], in1=t2[:], op=mybir.AluOpType.max
                )
                w_t = work.tile([P, H], bf16, tag="w")
                nc.scalar.activation(
                    out=w_t[:], in_=t3[:], func=mybir.ActivationFunctionType.Exp
                )
                # m = [w*z | w]
                m_t = work.tile([P, MW], bf16, tag="m")
                nc.vector.tensor_tensor(
                    out=_ap(m_t[:], 0, [[1, ZW]]),
                    in0=_ap(gb[:], gbase, [[1, ZW]]),
                    in1=_ap(w_t[:], 0, [[1, H], [0, D]]),
                    op=mybir.AluOpType.mult,
                )
                nc.vector.tensor_copy(out=_ap(m_t[:], ZW, [[1, H]]), in_=w_t[:])
                # aggregate
                if first:
                    agg_ps = psum.tile([P, MW], f32, tag="agg")
                nc.tensor.matmul(
                    out=agg_ps[:], lhsT=oh[:], rhs=m_t[:],
                    start=first, stop=last,
                )
                if last and phase == 0:
                    nc.vector.tensor_copy(
                        out=_ap(park[:], g * MW, [[1, MW]]), in_=agg_ps[:]
                    )
                if last and phase == 1:
                    tot = work.tile([P, MW], f32, tag="tot")
                    nc.vector.tensor_tensor(
                        out=tot[:],
                        in0=agg_ps[:],
                        in1=_ap(park[:], g * MW, [[1, MW]]),
                        op=mybir.AluOpType.add,
                    )
                    sden = work.tile([P, H], f32, tag="sden")
                    nc.vector.tensor_scalar_add(sden[:], tot[:, ZW:MW], 1e-30)
                    rs = work.tile([P, H], f32, tag="rs")
                    nc.vector.reciprocal(rs[:], sden[:])
                    h1 = work.tile([P, ZW], f32, tag="h1")
                    nc.vector.tensor_tensor(
                        out=h1[:],
                        in0=tot[:, :ZW],
                        in1=_ap(rs[:], 0, [[1, H], [0, D]]),
                        op=mybir.AluOpType.mult,
                    )
                    if elu:
                        pos = work.tile([P, ZW], f32, tag="pos")
                        nc.vector.tensor_scalar_max(pos[:], h1[:], 0.0)
                        ngx = work.tile([P, ZW], f32, tag="ngx")
                        nc.vector.tensor_scalar_min(ngx[:], h1[:], 0.0)
                        ex = work.tile([P, ZW], f32, tag="ex")
                        nc.scalar.activation(
                            out=ex[:], in_=ngx[:],
                            func=mybir.ActivationFunctionType.Exp,
                        )
                        hf = work.tile([P, ZW], f32, tag="hf")
                        nc.vector.tensor_tensor(
                            out=hf[:], in0=pos[:], in1=ex[:],
                            op=mybir.AluOpType.add,
                        )
                        out_t = work.tile([P, ZW], f32, tag="outt")
                        nc.vector.tensor_scalar_add(out_t[:], hf[:], -1.0)
                    else:
                        out_t = h1
                    nc.sync.dma_start(
                        out=OUT[g * P : (g + 1) * P, :], in_=out_t[:]
                    )
    mybir.codegen_inst_isa_subclasses(nc)
    return nc




# ------------------------------------------------- edge nc (batched DVE)
def build_edge_nc_v4b(plan, RWE, H, D, elu, n=N, npc=NPC, split=SPLIT):
    """Like build_edge_nc_v4 but DVE/ACT work batched per gather buffer
    (up to GCOLS columns per instruction) to amortize instruction
    dispatch overhead.  PE matmuls stay per column."""
    bass, tile, mybir = _bass_mods()
    from contextlib import ExitStack
    from concourse.library_config import mlp

    f32 = mybir.dt.float32
    bf16 = mybir.dt.bfloat16
    i16 = mybir.dt.int16

    ZW = H * D
    MW = ZW + H
    ng = plan["ng"]
    ncA, ncB = plan["ncA"], plan["ncB"]
    CA, CB = plan["CA"], plan["CB"]
    chunks = plan["chunks"]

    nc = bass.Bass("TRN2", num_swdge_queues=NQ)
    T = nc.dram_tensor("tbl", [n, RWE], bf16, kind="ExternalInput")
    IDXA = nc.dram_tensor("idxa", [P, CA * 8], i16, kind="ExternalInput")
    IDXB = nc.dram_tensor("idxb", [P, CB * 8], i16, kind="ExternalInput")
    DSL = nc.dram_tensor("dsl", [P, CA + CB], bf16, kind="ExternalInput")
    EDC = nc.dram_tensor("edc", [ng * P, H], bf16, kind="ExternalInput")
    IOTA = nc.dram_tensor("iota", [P, P], bf16, kind="ExternalInput")
    IDENT = nc.dram_tensor("ident", [P, P], bf16, kind="ExternalInput")
    OUT = nc.dram_tensor("out", [ng * P, ZW], f32, kind="ExternalOutput")

    colmap = []
    for phase, ncX in ((0, ncA), (1, ncB)):
        for g in range(ng):
            for k in range(ncX[g]):
                colmap.append((phase, g, k == 0, k == ncX[g] - 1))

    with _safe_tile_context()(nc) as tc:
        with ExitStack() as ctx:
            nc.gpsimd.load_library(mlp)
            const = ctx.enter_context(tc.tile_pool(name="const", bufs=1))
            gath = ctx.enter_context(tc.tile_pool(name="gath", bufs=6))
            work = ctx.enter_context(tc.tile_pool(name="work", bufs=3))
            psum = ctx.enter_context(
                tc.tile_pool(name="psum", bufs=2, space="PSUM")
            )
            psT = ctx.enter_context(
                tc.tile_pool(name="psT", bufs=3, space="PSUM")
            )

            iota = const.tile([P, P], bf16)
            nc.sync.dma_start(out=iota[:], in_=IOTA[:, :])
            ident = const.tile([P, P], bf16)
            nc.sync.dma_start(out=ident[:], in_=IDENT[:, :])
            idxa = const.tile([P, CA * 8], i16)
            nc.sync.dma_start(out=idxa[:], in_=IDXA[:, :])
            idxb = const.tile([P, CB * 8], i16)
            nc.sync.dma_start(out=idxb[:], in_=IDXB[:, :])
            dsl = const.tile([P, CA + CB], bf16)
            nc.sync.dma_start(out=dsl[:], in_=DSL[:, :])
            park = const.tile([P, ng * MW], f32)
            edg_all = const.tile([P, ng * H], bf16)
            for g in range(ng):
                nc.sync.dma_start(
                    out=edg_all[:, g * H : (g + 1) * H],
                    in_=EDC[g * P : (g + 1) * P, :],
                )

            for _ in range(6):
                gb = gath.tile([P, GCOLS * RWE], bf16, tag="gt")
                nc.vector.memset(gb[:], 0.0)

            ni_reg = nc.gpsimd.to_reg(GTOK)

            agg_ps = None
            ci = 0
            for gi, (phase, t0, nt) in enumerate(chunks):
                W8 = nt // P          # columns in this buffer
                idxt = idxa if phase == 0 else idxb
                gb = gath.tile([P, GCOLS * RWE], bf16, tag="gt")
                src_t = T[:split, :] if phase == 0 else T[split:, :]
                reg = ni_reg if nt == GTOK else nt
                nc.gpsimd.dma_gather(
                    _ap(gb[:], 0, [[RWE, W8], [1, RWE]]),
                    src_t,
                    idxt[:, t0 // 16 : t0 // 16 + nt // 16],
                    nt,
                    reg,
                    RWE,
                    queue_num=gi % NQ,
                )

                # ---- batched per-buffer work
                oh_b = work.tile([P, GCOLS * P], bf16, tag="oh")
                nc.vector.tensor_tensor(
                    out=_ap(oh_b[:], 0, [[P, W8], [1, P]]),
                    in0=_ap(dsl[:], ci, [[1, W8], [0, P]]),
                    in1=_ap(iota[:], 0, [[0, W8], [1, P]]),
                    op=mybir.AluOpType.is_equal,
                )
                # per-token ed via PE transpose + matmul (per column)
                psed_b = psT.tile([P, GCOLS * H], f32, tag="ed")
                ohT_b = work.tile([P, GCOLS * P], bf16, tag="ohTs")
                for j in range(W8):
                    pst = psT.tile([P, P], f32, tag="ohT")
                    nc.tensor.matmul(
                        out=pst[:],
                        lhsT=oh_b[:, j * P : (j + 1) * P],
                        rhs=ident[:],
                        start=True,
                        stop=True,
                    )
                    nc.scalar.activation(
                        out=ohT_b[:, j * P : (j + 1) * P], in_=pst[:],
                        func=mybir.ActivationFunctionType.Copy,
                    )
                    _, g, _, _ = colmap[ci + j]
                    nc.tensor.matmul(
                        out=psed_b[:, j * H : (j + 1) * H],
                        lhsT=ohT_b[:, j * P : (j + 1) * P],
                        rhs=edg_all[:, g * H : (g + 1) * H],
                        start=True,
                        stop=True,
                    )
                # w = exp(leakyrelu(es + ed)), batched
                es_f = work.tile([P, GCOLS * H], f32, tag="esf")
                nc.vector.tensor_copy(
                    out=_ap(es_f[:], 0, [[H, W8], [1, H]]),
                    in_=_ap(gb[:], ZW, [[RWE, W8], [1, H]]),
                )
                e_t = work.tile([P, GCOLS * H], f32, tag="e")
                nc.vector.tensor_tensor(
                    out=_ap(e_t[:], 0, [[H, W8], [1, H]]),
                    in0=_ap(es_f[:], 0, [[H, W8], [1, H]]),
                    in1=_ap(psed_b[:], 0, [[H, W8], [1, H]]),
                    op=mybir.AluOpType.add,
                )
                EC = W8 * H
                t2 = work.tile([P, GCOLS * H], f32, tag="t2")
                nc.vector.tensor_scalar_mul(t2[:, :EC], e_t[:, :EC], NEG_SLOPE)
                t3 = work.tile([P, GCOLS * H], f32, tag="t3")
                nc.vector.tensor_tensor(
                    out=t3[:, :EC], in0=e_t[:, :EC], in1=t2[:, :EC],
                    op=mybir.AluOpType.max,
                )
                w_b = work.tile([P, GCOLS * H], bf16, tag="w")
                nc.scalar.activation(
                    out=w_b[:, :EC], in_=t3[:, :EC],
                    func=mybir.ActivationFunctionType.Exp,
                )
                # m = [w*z | w], batched
                m_b = work.tile([P, GCOLS * MW], bf16, tag="m")
                nc.vector.tensor_tensor(
                    out=_ap(m_b[:], 0, [[MW, W8], [1, ZW]]),
                    in0=_ap(gb[:], 0, [[RWE, W8], [1, ZW]]),
                    in1=_ap(w_b[:], 0, [[H, W8], [1, H], [0, D]]),
                    op=mybir.AluOpType.mult,
                )
                nc.vector.tensor_copy(
                    out=_ap(m_b[:], ZW, [[MW, W8], [1, H]]),
                    in_=w_b[:, :EC],
                )
                # ---- aggregation + epilogue per column
                for j in range(W8):
                    c = ci + j
                    _, g, first, last = colmap[c]
                    if first:
                        agg_ps = psum.tile([P, MW], f32, tag="agg")
                    nc.tensor.matmul(
                        out=agg_ps[:],
                        lhsT=oh_b[:, j * P : (j + 1) * P],
                        rhs=m_b[:, j * MW : (j + 1) * MW],
                        start=first,
                        stop=last,
                    )
                    if last and phase == 0:
                        nc.vector.tensor_copy(
                            out=_ap(park[:], g * MW, [[1, MW]]), in_=agg_ps[:]
                        )
                    if last and phase == 1:
                        tot = work.tile([P, MW], f32, tag="tot")
                        nc.vector.tensor_tensor(
                            out=tot[:],
                            in0=agg_ps[:],
                            in1=_ap(park[:], g * MW, [[1, MW]]),
                            op=mybir.AluOpType.add,
                        )
                        sden = work.tile([P, H], f32, tag="sden")
                        nc.vector.tensor_scalar_add(
                            sden[:], tot[:, ZW:MW], 1e-30
                        )
                        rs = work.tile([P, H], f32, tag="rs")
                        nc.vector.reciprocal(rs[:], sden[:])
                        h1 = work.tile([P, ZW], f32, tag="h1")
                        nc.vector.tensor_tensor(
                            out=h1[:],
                            in0=tot[:, :ZW],
                            in1=_ap(rs[:], 0, [[1, H], [0, D]]),
                            op=mybir.AluOpType.mult,
                        )
                        if elu:
                            pos = work.tile([P, ZW], f32, tag="pos")
                            nc.vector.tensor_scalar_max(pos[:], h1[:], 0.0)
                            ngx = work.tile([P, ZW], f32, tag="ngx")
                            nc.vector.tensor_scalar_min(ngx[:], h1[:], 0.0)
                            ex = work.tile([P, ZW], f32, tag="ex")
                            nc.scalar.activation(
                                out=ex[:], in_=ngx[:],
                                func=mybir.ActivationFunctionType.Exp,
                            )
                            hf = work.tile([P, ZW], f32, tag="hf")
                            nc.vector.tensor_tensor(
                                out=hf[:], in0=pos[:], in1=ex[:],
                                op=mybir.AluOpType.add,
                            )
                            out_t = work.tile([P, ZW], f32, tag="outt")
                            nc.vector.tensor_scalar_add(out_t[:], hf[:], -1.0)
                        else:
                            out_t = h1
                        nc.sync.dma_start(
                            out=OUT[g * P : (g + 1) * P, :], in_=out_t[:]
                        )
                ci += W8
    mybir.codegen_inst_isa_subclasses(nc)
    return nc


# ------------------------------------------------- edge nc v5 (host oh/edt)
def build_edge_nc_v5(plan, RWE, H, D, elu, n=N, npc=NPC, split=SPLIT):
    """v4b minus on-device one-hot build and per-token-ed machinery.

    OHS [128, ncols*128] bf16  host-built one-hot (streamed per buffer)
    EDT [128, ncols*H]   bf16  host-gathered ed of each token's dst
    """
    bass, tile, mybir = _bass_mods()
    from contextlib import ExitStack
    from concourse.library_config import mlp

    f32 = mybir.dt.float32
    bf16 = mybir.dt.bfloat16
    i16 = mybir.dt.int16

    ZW = H * D
    MW = ZW + H
    ng = plan["ng"]
    ncA, ncB = plan["ncA"], plan["ncB"]
    CA, CB = plan["CA"], plan["CB"]
    chunks = plan["chunks"]
    NC = CA + CB

    nc = bass.Bass("TRN2", num_swdge_queues=NQ)
    T = nc.dram_tensor("tbl", [n, RWE], bf16, kind="ExternalInput")
    IDXA = nc.dram_tensor("idxa", [P, CA * 8], i16, kind="ExternalInput")
    IDXB = nc.dram_tensor("idxb", [P, CB * 8], i16, kind="ExternalInput")
    OHS = nc.dram_tensor("ohs", [P, NC * P], bf16, kind="ExternalInput")
    EDT = nc.dram_tensor("edt", [P, NC * H], bf16, kind="ExternalInput")
    OUT = nc.dram_tensor("out", [ng * P, ZW], f32, kind="ExternalOutput")

    colmap = []
    for phase, ncX in ((0, ncA), (1, ncB)):
        for g in range(ng):
            for k in range(ncX[g]):
                colmap.append((phase, g, k == 0, k == ncX[g] - 1))

    with _safe_tile_context()(nc) as tc:
        with ExitStack() as ctx:
            nc.gpsimd.load_library(mlp)
            const = ctx.enter_context(tc.tile_pool(name="const", bufs=1))
            gath = ctx.enter_context(tc.tile_pool(name="gath", bufs=6))
            ohp = ctx.enter_context(tc.tile_pool(name="ohp", bufs=6))
            work = ctx.enter_context(tc.tile_pool(name="work", bufs=3))
            psum = ctx.enter_context(
                tc.tile_pool(name="psum", bufs=2, space="PSUM")
            )

            idxa = const.tile([P, CA * 8], i16)
            nc.sync.dma_start(out=idxa[:], in_=IDXA[:, :])
            idxb = const.tile([P, CB * 8], i16)
            nc.sync.dma_start(out=idxb[:], in_=IDXB[:, :])
            edt = const.tile([P, NC * H], bf16)
            nc.sync.dma_start(out=edt[:], in_=EDT[:, :])
            park = const.tile([P, ng * MW], f32)

            for _ in range(6):
                gb = gath.tile([P, GCOLS * RWE], bf16, tag="gt")
                nc.vector.memset(gb[:], 0.0)

            ni_reg = nc.gpsimd.to_reg(GTOK)

            agg_ps = None
            ci = 0
            for gi, (phase, t0, nt) in enumerate(chunks):
                W8 = nt // P
                idxt = idxa if phase == 0 else idxb
                gb = gath.tile([P, GCOLS * RWE], bf16, tag="gt")
                src_t = T[:split, :] if phase == 0 else T[split:, :]
                reg = ni_reg if nt == GTOK else nt
                nc.gpsimd.dma_gather(
                    _ap(gb[:], 0, [[RWE, W8], [1, RWE]]),
                    src_t,
                    idxt[:, t0 // 16 : t0 // 16 + nt // 16],
                    nt,
                    reg,
                    RWE,
                    queue_num=gi % NQ,
                )
                oh_b = ohp.tile([P, GCOLS * P], bf16, tag="oh")
                nc.sync.dma_start(
                    out=oh_b[:, : W8 * P],
                    in_=OHS[:, ci * P : (ci + W8) * P],
                )

                # w = exp(leakyrelu(es + ed)), batched per buffer
                e_t = work.tile([P, GCOLS * H], f32, tag="e")
                nc.vector.tensor_tensor(
                    out=_ap(e_t[:], 0, [[H, W8], [1, H]]),
                    in0=_ap(gb[:], ZW, [[RWE, W8], [1, H]]),
                    in1=_ap(edt[:], ci * H, [[H, W8], [1, H]]),
                    op=mybir.AluOpType.add,
                )
                EC = W8 * H
                t2 = work.tile([P, GCOLS * H], f32, tag="t2")
                nc.vector.tensor_scalar_mul(t2[:, :EC], e_t[:, :EC], NEG_SLOPE)
                t3 = work.tile([P, GCOLS * H], f32, tag="t3")
                nc.vector.tensor_tensor(
                    out=t3[:, :EC], in0=e_t[:, :EC], in1=t2[:, :EC],
                    op=mybir.AluOpType.max,
                )
                w_b = work.tile([P, GCOLS * H], bf16, tag="w")
                nc.scalar.activation(
                    out=w_b[:, :EC], in_=t3[:, :EC],
                    func=mybir.ActivationFunctionType.Exp,
                )
                m_b = work.tile([P, GCOLS * MW], bf16, tag="m")
                nc.vector.tensor_tensor(
                    out=_ap(m_b[:], 0, [[MW, W8], [1, ZW]]),
                    in0=_ap(gb[:], 0, [[RWE, W8], [1, ZW]]),
                    in1=_ap(w_b[:], 0, [[H, W8], [1, H], [0, D]]),
                    op=mybir.AluOpType.mult,
                )
                nc.vector.tensor_copy(
                    out=_ap(m_b[:], ZW, [[MW, W8], [1, H]]),
                    in_=w_b[:, :EC],
                )
                for j in range(W8):
                    c = ci + j
                    _, g, first, last = colmap[c]
                    if first:
                        agg_ps = psum.tile([P, MW], f32, tag="agg")
                    nc.tensor.matmul(
                        out=agg_ps[:],
                        lhsT=oh_b[:, j * P : (j + 1) * P],
                        rhs=m_b[:, j * MW : (j + 1) * MW],
                        start=first,
                        stop=last,
                    )
                    if last and phase == 0:
                        nc.vector.tensor_copy(
                            out=_ap(park[:], g * MW, [[1, MW]]), in_=agg_ps[:]
                        )
                    if last and phase == 1:
                        tot = work.tile([P, MW], f32, tag="tot")
                        nc.vector.tensor_tensor(
                            out=tot[:],
                            in0=agg_ps[:],
                            in1=_ap(park[:], g * MW, [[1, MW]]),
                            op=mybir.AluOpType.add,
                        )
                        sden = work.tile([P, H], f32, tag="sden")
                        nc.vector.tensor_scalar_add(
                            sden[:], tot[:, ZW:MW], 1e-30
                        )
                        rs = work.tile([P, H], f32, tag="rs")
                        nc.vector.reciprocal(rs[:], sden[:])
                        h1 = work.tile([P, ZW], f32, tag="h1")
                        nc.vector.tensor_tensor(
                            out=h1[:],
                            in0=tot[:, :ZW],
                            in1=_ap(rs[:], 0, [[1, H], [0, D]]),
                            op=mybir.AluOpType.mult,
                        )
                        if elu:
                            pos = work.tile([P, ZW], f32, tag="pos")
                            nc.vector.tensor_scalar_max(pos[:], h1[:], 0.0)
                            ngx = work.tile([P, ZW], f32, tag="ngx")
                            nc.vector.tensor_scalar_min(ngx[:], h1[:], 0.0)
                            ex = work.tile([P, ZW], f32, tag="ex")
                            nc.scalar.activation(
                                out=ex[:], in_=ngx[:],
                                func=mybir.ActivationFunctionType.Exp,
                            )
                            hf = work.tile([P, ZW], f32, tag="hf")
                            nc.vector.tensor_tensor(
                                out=hf[:], in0=pos[:], in1=ex[:],
                                op=mybir.AluOpType.add,
                            )
                            out_t = work.tile([P, ZW], f32, tag="outt")
                            nc.vector.tensor_scalar_add(out_t[:], hf[:], -1.0)
                        else:
                            out_t = h1
                        nc.sync.dma_start(
                            out=OUT[g * P : (g + 1) * P, :], in_=out_t[:]
                        )
                ci += W8
    mybir.codegen_inst_isa_subclasses(nc)
    return nc


# ------------------------------------------------- edge nc v6 (z-only rows)
def build_edge_nc_v6(plan, RWE, H, D, elu, n=N, npc=NPC, split=SPLIT):
    """v5 with z-only gathered rows (es folded into the host-built
    e_tok = es[src]+ed[dst] array), contiguous message layout, and the
    denominator via a second small matmul.  `elu` is ignored here (ELU
    is applied by the next dense kernel)."""
    bass, tile, mybir = _bass_mods()
    from contextlib import ExitStack
    from concourse.library_config import mlp

    f32 = mybir.dt.float32
    bf16 = mybir.dt.bfloat16
    i16 = mybir.dt.int16

    ZW = H * D
    MW = ZW + H
    ng = plan["ng"]
    ncA, ncB = plan["ncA"], plan["ncB"]
    CA, CB = plan["CA"], plan["CB"]
    chunks = plan["chunks"]
    NC = CA + CB

    nc = bass.Bass("TRN2", num_swdge_queues=NQ)
    T = nc.dram_tensor("tbl", [n, RWE], bf16, kind="ExternalInput")
    IDXA = nc.dram_tensor("idxa", [P, CA * 8], i16, kind="ExternalInput")
    IDXB = nc.dram_tensor("idxb", [P, CB * 8], i16, kind="ExternalInput")
    OHS = nc.dram_tensor("ohs", [P, NC * P], bf16, kind="ExternalInput")
    ETOK = nc.dram_tensor("etok", [P, NC * H], bf16, kind="ExternalInput")
    OUT = nc.dram_tensor("out", [ng * P, ZW], f32, kind="ExternalOutput")

    colmap = []
    for phase, ncX in ((0, ncA), (1, ncB)):
        for g in range(ng):
            for k in range(ncX[g]):
                colmap.append((phase, g, k == 0, k == ncX[g] - 1))

    with _safe_tile_context()(nc) as tc:
        with ExitStack() as ctx:
            nc.gpsimd.load_library(mlp)
            const = ctx.enter_context(tc.tile_pool(name="const", bufs=1))
            gath = ctx.enter_context(tc.tile_pool(name="gath", bufs=6))
            ohp = ctx.enter_context(tc.tile_pool(name="ohp", bufs=6))
            work = ctx.enter_context(tc.tile_pool(name="work", bufs=3))
            psum = ctx.enter_context(
                tc.tile_pool(name="psum", bufs=2, space="PSUM")
            )

            idxa = const.tile([P, CA * 8], i16)
            nc.sync.dma_start(out=idxa[:], in_=IDXA[:, :])
            idxb = const.tile([P, CB * 8], i16)
            nc.sync.dma_start(out=idxb[:], in_=IDXB[:, :])
            etok = const.tile([P, NC * H], bf16)
            nc.sync.dma_start(out=etok[:], in_=ETOK[:, :])
            park = const.tile([P, ng * MW], f32)

            for _ in range(6):
                gb = gath.tile([P, GCOLS * RWE], bf16, tag="gt")
                nc.vector.memset(gb[:], 0.0)

            ni_reg = nc.gpsimd.to_reg(GTOK)

            agg_ps = None
            ci = 0
            for gi, (phase, t0, nt) in enumerate(chunks):
                W8 = nt // P
                idxt = idxa if phase == 0 else idxb
                gb = gath.tile([P, GCOLS * RWE], bf16, tag="gt")
                src_t = T[:split, :] if phase == 0 else T[split:, :]
                reg = ni_reg if nt == GTOK else nt
                nc.gpsimd.dma_gather(
                    _ap(gb[:], 0, [[RWE, W8], [1, RWE]]),
                    src_t,
                    idxt[:, t0 // 16 : t0 // 16 + nt // 16],
                    nt,
                    reg,
                    RWE,
                    queue_num=gi % NQ,
                )
                oh_b = ohp.tile([P, GCOLS * P], bf16, tag="oh")
                nc.sync.dma_start(
                    out=oh_b[:, : W8 * P],
                    in_=OHS[:, ci * P : (ci + W8) * P],
                )

                # w = exp(leakyrelu(e_tok)); all slices contiguous
                EC = W8 * H
                e_sl = etok[:, ci * H : ci * H + EC]
                t2 = work.tile([P, GCOLS * H], f32, tag="t2")
                nc.vector.tensor_scalar_mul(t2[:, :EC], e_sl, NEG_SLOPE)
                t3 = work.tile([P, GCOLS * H], f32, tag="t3")
                nc.vector.tensor_tensor(
                    out=t3[:, :EC], in0=e_sl, in1=t2[:, :EC],
                    op=mybir.AluOpType.max,
                )
                w_b = work.tile([P, GCOLS * H], bf16, tag="w")
                nc.scalar.activation(
                    out=w_b[:, :EC], in_=t3[:, :EC],
                    func=mybir.ActivationFunctionType.Exp,
                )
                # m = w*z, contiguous out (gb rows are pure z)
                m_b = work.tile([P, GCOLS * ZW], bf16, tag="m")
                nc.vector.tensor_tensor(
                    out=m_b[:, : W8 * ZW],
                    in0=_ap(gb[:], 0, [[RWE, W8], [1, ZW]]),
                    in1=_ap(w_b[:], 0, [[H, W8], [1, H], [0, D]]),
                    op=mybir.AluOpType.mult,
                )
                for j in range(W8):
                    c = ci + j
                    _, g, first, last = colmap[c]
                    if first:
                        agg_ps = psum.tile([P, ZW], f32, tag="agg")
                        den_ps = psum.tile([P, H], f32, tag="den")
                    nc.tensor.matmul(
                        out=agg_ps[:],
                        lhsT=oh_b[:, j * P : (j + 1) * P],
                        rhs=m_b[:, j * ZW : (j + 1) * ZW],
                        start=first,
                        stop=last,
                    )
                    nc.tensor.matmul(
                        out=den_ps[:],
                        lhsT=oh_b[:, j * P : (j + 1) * P],
                        rhs=w_b[:, j * H : (j + 1) * H],
                        start=first,
                        stop=last,
                    )
                    if last and phase == 0:
                        nc.vector.tensor_copy(
                            out=_ap(park[:], g * MW, [[1, ZW]]), in_=agg_ps[:]
                        )
                        nc.vector.tensor_copy(
                            out=_ap(park[:], g * MW + ZW, [[1, H]]),
                            in_=den_ps[:],
                        )
                    if last and phase == 1:
                        totz = work.tile([P, ZW], f32, tag="totz")
                        nc.vector.tensor_tensor(
                            out=totz[:],
                            in0=agg_ps[:],
                            in1=_ap(park[:], g * MW, [[1, ZW]]),
                            op=mybir.AluOpType.add,
                        )
                        totd = work.tile([P, H], f32, tag="totd")
                        nc.vector.tensor_tensor(
                            out=totd[:],
                            in0=den_ps[:],
                            in1=_ap(park[:], g * MW + ZW, [[1, H]]),
                            op=mybir.AluOpType.add,
                        )
                        sden = work.tile([P, H], f32, tag="sden")
                        nc.vector.tensor_scalar_add(sden[:], totd[:], 1e-30)
                        rs = work.tile([P, H], f32, tag="rs")
                        nc.vector.reciprocal(rs[:], sden[:])
                        h1 = work.tile([P, ZW], f32, tag="h1")
                        nc.vector.tensor_tensor(
                            out=h1[:],
                            in0=totz[:],
                            in1=_ap(rs[:], 0, [[1, H], [0, D]]),
                            op=mybir.AluOpType.mult,
                        )
                        nc.sync.dma_start(
                            out=OUT[g * P : (g + 1) * P, :], in_=h1[:]
                        )
                ci += W8
    mybir.codegen_inst_isa_subclasses(nc)
    return nc


# ------------------------------------------------- plan v7 (indirect DMA)
def build_plan_v7(src, dst, n=N, cores=CORES, npc=NPC):
    """Single-phase token streams (int32 indices, no A/B split).

    Tokens grouped by dst group only; idx32 [P, NC] per core; host-built
    one-hot stream OHS [P, NC*P]; tokdst/toksrc for etok.
    """
    key = ("v7", src.tobytes(), dst.tobytes(), n, cores, npc)
    h = hash(key)
    if h in _PLAN_CACHE:
        return _PLAN_CACHE[h]
    import ml_dtypes

    ng = (npc + P - 1) // P
    order = np.argsort(dst, kind="stable")
    ssrc = src[order].astype(np.int64)
    sdst = dst[order].astype(np.int64)
    deg = np.bincount(dst, minlength=n).astype(np.int64)
    starts = np.zeros(n + 1, dtype=np.int64)
    np.cumsum(deg, out=starts[1:])

    eg = [[None] * ng for _ in range(cores)]
    for c in range(cores):
        base = c * npc
        for g in range(ng):
            lo = base + g * P
            hi = min(base + (g + 1) * P, base + npc)
            es_ = ssrc[starts[lo]:starts[hi]]
            ds_ = sdst[starts[lo]:starts[hi]]
            o = np.argsort(es_, kind="stable")
            eg[c][g] = (es_[o], ds_[o] - lo)

    ncX = [max(1, max((len(eg[c][g][0]) + P - 1) // P for c in range(cores)))
           for g in range(ng)]
    NC = sum(ncX)

    cores_arr = []
    for c in range(cores):
        toksrc = np.zeros((P, NC), dtype=np.int64)
        dsl = np.full((P, NC), -1.0, dtype=np.float32)
        col = 0
        for g in range(ng):
            es_, dslot = eg[c][g]
            ne = len(es_)
            j = np.arange(ne)
            toksrc[j % P, col + j // P] = es_
            dsl[j % P, col + j // P] = dslot
            col += ncX[g]
        oh = (dsl[:, :, None] == np.arange(P, dtype=np.float32)[None, None, :])
        oh = np.ascontiguousarray(
            oh.reshape(P, NC * P)).astype(ml_dtypes.bfloat16)
        gcol = np.zeros(NC, dtype=np.int64)
        col = 0
        for g in range(ng):
            gcol[col:col + ncX[g]] = g
            col += ncX[g]
        tokdst = np.where(dsl >= 0, gcol[None, :] * P + dsl, -1.0
                          ).astype(np.int64)
        cores_arr.append({
            "idx32": np.ascontiguousarray(toksrc.astype(np.int32)),
            "oh": oh,
            "tokdst": tokdst,
            "toksrc": toksrc,
        })

    colmap = []
    for g in range(ng):
        for k in range(ncX[g]):
            colmap.append((g, k == 0, k == ncX[g] - 1))

    plan = {"ng": ng, "ncX": ncX, "NC": NC, "colmap": colmap,
            "cores": cores_arr}
    edge_tot = sum(len(eg[c][g][0]) for c in range(cores)
                   for g in range(ng)) / cores
    plan["pad_frac"] = NC * P / max(edge_tot, 1) - 1.0
    _PLAN_CACHE[h] = plan
    return plan


# ------------------------------------------------- edge nc v7 (indirect)
def build_edge_nc_v7(plan, RWE, H, D, n=N, kk=8, nbuf=8):
    """Edge kernel using indirect-DMA gathers (one chunk = kk columns).

    T    [n, RWE]    bf16  row = z (H*D), rest pad
    IDX  [128, NC]   i32   token t=(c*128+p) -> src at [p, c]
    OHS  [128, NC*P] bf16  host-built one-hot stream
    ETOK [128, NC*H] bf16  es[src]+ed[dst] per token (-1e4 pads)
    OUT  [ng*128, H*D] f32 aggregated z (pre-ELU), denominator-normalized
    """
    bass, tile, mybir = _bass_mods()
    from contextlib import ExitStack

    f32 = mybir.dt.float32
    bf16 = mybir.dt.bfloat16
    i32 = mybir.dt.int32

    ZW = H * D
    ng = plan["ng"]
    NC = plan["NC"]
    colmap = plan["colmap"]

    nc = bass.Bass("TRN2")
    T = nc.dram_tensor("tbl", [n, RWE], bf16, kind="ExternalInput")
    IDX = nc.dram_tensor("idx32", [P, NC], i32, kind="ExternalInput")
    OHS = nc.dram_tensor("ohs", [P, NC * P], bf16, kind="ExternalInput")
    ETOK = nc.dram_tensor("etok", [P, NC * H], bf16, kind="ExternalInput")
    OUT = nc.dram_tensor("out", [ng * P, ZW], f32, kind="ExternalOutput")

    chunks = []
    c0 = 0
    while c0 < NC:
        chunks.append((c0, min(kk, NC - c0)))
        c0 += kk

    with _safe_tile_context()(nc) as tc:
        with ExitStack() as ctx:
            const = ctx.enter_context(tc.tile_pool(name="const", bufs=1))
            gath = ctx.enter_context(tc.tile_pool(name="gath", bufs=nbuf))
            ohp = ctx.enter_context(tc.tile_pool(name="ohp", bufs=nbuf))
            work = ctx.enter_context(tc.tile_pool(name="work", bufs=4))
            psum = ctx.enter_context(
                tc.tile_pool(name="psum", bufs=2, space="PSUM")
            )

            idx = const.tile([P, NC], i32)
            nc.sync.dma_start(out=idx[:], in_=IDX[:, :])
            etok = const.tile([P, NC * H], bf16)
            nc.sync.dma_start(out=etok[:], in_=ETOK[:, :])

            for _ in range(nbuf):
                gb = gath.tile([P, kk * RWE], bf16, tag="gt")
                nc.vector.memset(gb[:], 0.0)

            agg_ps = None
            for gi, (c0, cw) in enumerate(chunks):
                gb = gath.tile([P, kk * RWE], bf16, tag="gt")
                nc.gpsimd.indirect_dma_start(
                    out=gb[:, : cw * RWE],
                    out_offset=None,
                    in_=T[:, :],
                    in_offset=bass.IndirectOffsetOnAxis(
                        ap=idx[:, c0: c0 + cw], axis=0
                    ),
                )
                oh_b = ohp.tile([P, kk * P], bf16, tag="oh")
                oh_eng = nc.sync if gi % 2 == 0 else nc.scalar
                oh_eng.dma_start(
                    out=oh_b[:, : cw * P],
                    in_=OHS[:, c0 * P: (c0 + cw) * P],
                )

                # w = exp(lrelu(e)) = max(exp(e), exp(0.2*e)) — both exps
                # on the (idle) ACT engine, one small max on DVE
                EC = cw * H
                e_sl = etok[:, c0 * H: c0 * H + EC]
                ea = work.tile([P, kk * H], f32, tag="ea")
                nc.scalar.activation(
                    out=ea[:, :EC], in_=e_sl,
                    func=mybir.ActivationFunctionType.Exp,
                )
                eb = work.tile([P, kk * H], f32, tag="eb")
                nc.scalar.activation(
                    out=eb[:, :EC], in_=e_sl,
                    func=mybir.ActivationFunctionType.Exp,
                    scale=NEG_SLOPE,
                )
                w_b = work.tile([P, kk * H], bf16, tag="w")
                nc.vector.tensor_tensor(
                    out=w_b[:, :EC], in0=ea[:, :EC], in1=eb[:, :EC],
                    op=mybir.AluOpType.max,
                )
                # m = w*z  (layer1 table is d-major so w broadcast is
                # innermost-contiguous; layer2 H=1 stays d-major trivially)
                m_b = work.tile([P, kk * ZW], bf16, tag="m")
                if H > 1:
                    in1_dims = [[H, cw], [0, D], [1, H]]
                else:
                    in1_dims = [[H, cw], [0, ZW]]
                nc.vector.tensor_tensor(
                    out=m_b[:, : cw * ZW],
                    in0=_ap(gb[:], 0, [[RWE, cw], [1, ZW]]),
                    in1=_ap(w_b[:], 0, in1_dims),
                    op=mybir.AluOpType.mult,
                )
                for j in range(cw):
                    c = c0 + j
                    g, first, last = colmap[c]
                    if first:
                        agg_ps = psum.tile([P, ZW], f32, tag="agg")
                        den_ps = psum.tile([P, H], f32, tag="den")
                    nc.tensor.matmul(
                        out=agg_ps[:],
                        lhsT=oh_b[:, j * P: (j + 1) * P],
                        rhs=m_b[:, j * ZW: (j + 1) * ZW],
                        start=first, stop=last,
                    )
                    nc.tensor.matmul(
                        out=den_ps[:],
                        lhsT=oh_b[:, j * P: (j + 1) * P],
                        rhs=w_b[:, j * H: (j + 1) * H],
                        start=first, stop=last,
                    )
                    if last:
                        sden = work.tile([P, H], f32, tag="sden")
                        nc.vector.tensor_scalar_add(sden[:], den_ps[:], 1e-30)
                        rs = work.tile([P, H], f32, tag="rs")
                        nc.vector.reciprocal(rs[:], sden[:])
                        h1 = work.tile([P, ZW], f32, tag="h1")
                        rs_dims = [[0, D], [1, H]] if H > 1 else [[0, ZW]]
                        nc.vector.tensor_tensor(
                            out=h1[:],
                            in0=agg_ps[:],
                            in1=_ap(rs[:], 0, rs_dims),
                            op=mybir.AluOpType.mult,
                        )
                        nc.sync.dma_start(
                            out=OUT[g * P: (g + 1) * P, :], in_=h1[:]
                        )
    mybir.codegen_inst_isa_subclasses(nc)
    return nc


# ------------------------------------------------- edge nc v8 (swdge+ACT)
def build_edge_nc_v8(plan, RWE, H, D, n=N, npc=NPC, split=SPLIT):
    """v6 pipeline with DVE offload: attention exps on ACT, d-major m_b
    broadcast (layer 1), park copies on ACT, 4 PSUM buffers, OHS loads
    alternating over the sync/scalar HW queues."""
    bass, tile, mybir = _bass_mods()
    from contextlib import ExitStack
    from concourse.library_config import mlp

    f32 = mybir.dt.float32
    bf16 = mybir.dt.bfloat16
    i16 = mybir.dt.int16

    ZW = H * D
    MW = ZW + H
    ng = plan["ng"]
    ncA, ncB = plan["ncA"], plan["ncB"]
    CA, CB = plan["CA"], plan["CB"]
    chunks = plan["chunks"]
    NC = CA + CB

    nc = bass.Bass("TRN2", num_swdge_queues=NQ)
    T = nc.dram_tensor("tbl", [n, RWE], bf16, kind="ExternalInput")
    IDXA = nc.dram_tensor("idxa", [P, CA * 8], i16, kind="ExternalInput")
    IDXB = nc.dram_tensor("idxb", [P, CB * 8], i16, kind="ExternalInput")
    f8 = mybir.dt.float8e4
    OHS = nc.dram_tensor("ohs", [P, NC * P], f8, kind="ExternalInput")
    ETOK = nc.dram_tensor("etok", [P, NC * H], bf16, kind="ExternalInput")
    IDENT = nc.dram_tensor("ident", [P, P], bf16, kind="ExternalInput")
    OUT = nc.dram_tensor("out", [ng * P, ZW], f32, kind="ExternalOutput")

    colmap = []
    for phase, ncX in ((0, ncA), (1, ncB)):
        for g in range(ng):
            for k in range(ncX[g]):
                colmap.append((phase, g, k == 0, k == ncX[g] - 1))

    with _safe_tile_context()(nc) as tc:
        with ExitStack() as ctx:
            nc.gpsimd.load_library(mlp)
            const = ctx.enter_context(tc.tile_pool(name="const", bufs=1))
            gath = ctx.enter_context(tc.tile_pool(name="gath", bufs=10))
            ohp = ctx.enter_context(tc.tile_pool(name="ohp", bufs=10))
            work = ctx.enter_context(tc.tile_pool(name="work", bufs=4))
            psum = ctx.enter_context(
                tc.tile_pool(name="psum", bufs=6, space="PSUM")
            )

            idxa = const.tile([P, CA * 8], i16)
            nc.sync.dma_start(out=idxa[:], in_=IDXA[:, :])
            idxb = const.tile([P, CB * 8], i16)
            nc.sync.dma_start(out=idxb[:], in_=IDXB[:, :])
            etok = const.tile([P, NC * H], bf16)
            nc.sync.dma_start(out=etok[:], in_=ETOK[:, :])
            ident = const.tile([P, P], bf16)
            nc.sync.dma_start(out=ident[:], in_=IDENT[:, :])
            park = const.tile([P, ng * MW], bf16)

            for _ in range(10):
                gb = gath.tile([P, GCOLS * RWE], bf16, tag="gt")
                nc.vector.memset(gb[:], 0.0)

            ni_reg = nc.gpsimd.to_reg(GTOK)
            prep_trig = bool(int(os.environ.get("GAT_PREP_TRIG", "0")))
            dma_sems = [nc.alloc_semaphore(f"gsem{q}") for q in range(NQ)]

            agg_ps = None
            ci = 0
            for gi, (phase, t0, nt) in enumerate(chunks):
                W8 = nt // P
                idxt = idxa if phase == 0 else idxb
                gb = gath.tile([P, GCOLS * RWE], bf16, tag="gt")
                src_t = T[:split, :] if phase == 0 else T[split:, :]
                reg = ni_reg if nt == GTOK else nt
                q = gi % NQ
                if prep_trig:
                    # prep writes only the descriptor ring (engine never
                    # stalls on gather-buffer reuse); the trigger carries
                    # the deferred buffer deps and fires the DMA
                    nc.gpsimd.dma_gather(
                        _ap(gb[:], 0, [[RWE, W8], [1, RWE]]),
                        src_t,
                        idxt[:, t0 // 16: t0 // 16 + nt // 16],
                        nt, reg, RWE,
                        prepare_only=True, sem=dma_sems[q],
                        queue_num=q,
                    )
                    nc.gpsimd.trigger_dma(count=None, queue_num=q)
                else:
                    nc.gpsimd.dma_gather(
                        _ap(gb[:], 0, [[RWE, W8], [1, RWE]]),
                        src_t,
                        idxt[:, t0 // 16: t0 // 16 + nt // 16],
                        nt, reg, RWE,
                        queue_num=q,
                    )
                oh_b = ohp.tile([P, GCOLS * P], f8, tag="oh")
                oh_eng = nc.sync if gi % 2 == 0 else nc.scalar
                oh_eng.dma_start(
                    out=oh_b[:, : W8 * P],
                    in_=OHS[:, ci * P: (ci + W8) * P],
                )

                # w = exp(lrelu(e)) = max(exp(e), exp(0.2 e)); exps on ACT
                EC = W8 * H
                e_sl = etok[:, ci * H: ci * H + EC]
                ea = work.tile([P, GCOLS * H], f32, tag="ea")
                nc.scalar.activation(
                    out=ea[:, :EC], in_=e_sl,
                    func=mybir.ActivationFunctionType.Exp,
                )
                eb = work.tile([P, GCOLS * H], f32, tag="eb")
                nc.scalar.activation(
                    out=eb[:, :EC], in_=e_sl,
                    func=mybir.ActivationFunctionType.Exp,
                    scale=NEG_SLOPE,
                )
                w_b = work.tile([P, GCOLS * H], bf16, tag="w")
                nc.vector.tensor_tensor(
                    out=w_b[:, :EC], in0=ea[:, :EC], in1=eb[:, :EC],
                    op=mybir.AluOpType.max,
                )
                # m = [w*z | w] (z d-major for H>1: w broadcast contiguous)
                m_b = work.tile([P, GCOLS * MW], bf16, tag="m")
                if H > 1:
                    in1_dims = [[H, W8], [0, D], [1, H]]
                else:
                    in1_dims = [[H, W8], [0, ZW]]
                nc.vector.tensor_tensor(
                    out=_ap(m_b[:], 0, [[MW, W8], [1, ZW]]),
                    in0=_ap(gb[:], 0, [[RWE, W8], [1, ZW]]),
                    in1=_ap(w_b[:], 0, in1_dims),
                    op=mybir.AluOpType.mult,
                )
                nc.vector.tensor_copy(
                    out=_ap(m_b[:], ZW, [[MW, W8], [1, H]]),
                    in_=_ap(w_b[:], 0, [[H, W8], [1, H]]),
                )
                for j in range(W8):
                    c = ci + j
                    phase_c, g, first, last = colmap[c]
                    if first:
                        agg_ps = psum.tile([P, MW], f32, tag="agg")
                        if phase_c == 1:
                            # reinject parked phase-A partials via PE
                            nc.tensor.matmul(
                                out=agg_ps[:],
                                lhsT=ident[:],
                                rhs=_ap(park[:], g * MW, [[1, MW]]),
                                start=True, stop=False,
                            )
                    nc.tensor.matmul(
                        out=agg_ps[:],
                        lhsT=oh_b[:, j * P: (j + 1) * P],
                        rhs=m_b[:, j * MW: (j + 1) * MW],
                        start=(first and phase_c == 0), stop=last,
                    )
                    if last and phase_c == 0:
                        nc.scalar.activation(
                            out=_ap(park[:], g * MW, [[1, MW]]),
                            in_=agg_ps[:],
                            func=mybir.ActivationFunctionType.Copy,
                        )
                    if last and phase_c == 1:
                        sden = work.tile([P, H], f32, tag="sden")
                        nc.scalar.activation(
                            out=sden[:], in_=agg_ps[:, ZW:MW],
                            func=mybir.ActivationFunctionType.Copy,
                            bias=1e-30,
                        )
                        rs = work.tile([P, H], f32, tag="rs")
                        nc.vector.reciprocal(rs[:], sden[:])
                        h1 = work.tile([P, ZW], f32, tag="h1")
                        rs_dims = [[0, D], [1, H]] if H > 1 else [[0, ZW]]
                        nc.vector.tensor_tensor(
                            out=h1[:], in0=agg_ps[:, :ZW],
                            in1=_ap(rs[:], 0, rs_dims),
                            op=mybir.AluOpType.mult,
                        )
                        nc.sync.dma_start(
                            out=OUT[g * P: (g + 1) * P, :], in_=h1[:]
                        )
                ci += W8
    mybir.codegen_inst_isa_subclasses(nc)
    return nc


# ---------------------------------------------------------------- dense nc
def build_dense_nc(elu_in=False):
    """out[tile] = elu?(xT)[:, tile].T @ Waug -> [NT*P, DENSE_W] (fp32)."""
    bass, tile, mybir = _bass_mods()
    from contextlib import ExitStack

    f32 = mybir.dt.float32
    nc = bass.Bass("TRN2")
    xT = nc.dram_tensor("xt", [P, NT * P], f32, kind="ExternalInput")
    W = nc.dram_tensor("waug", [P, DENSE_W], f32, kind="ExternalInput")
    OUTD = nc.dram_tensor("outd", [NT * P, DENSE_W], f32, kind="ExternalOutput")

    with _safe_tile_context()(nc) as tc:
        with ExitStack() as ctx:
            const = ctx.enter_context(tc.tile_pool(name="const", bufs=1))
            work = ctx.enter_context(tc.tile_pool(name="work", bufs=3))
            psum = ctx.enter_context(tc.tile_pool(name="psum", bufs=4, space="PSUM"))

            wsb = const.tile([P, DENSE_W], f32)
            nc.sync.dma_start(out=wsb[:], in_=W[:, :])
            xsb = const.tile([P, NT * P], f32)
            nc.sync.dma_start(out=xsb[:], in_=xT[:, :])
            if elu_in:
                pos = const.tile([P, NT * P], f32)
                nc.vector.tensor_scalar_max(pos[:], xsb[:], 0.0)
                ngx = const.tile([P, NT * P], f32)
                nc.vector.tensor_scalar_min(ngx[:], xsb[:], 0.0)
                ex = const.tile([P, NT * P], f32)
                nc.scalar.activation(
                    out=ex[:], in_=ngx[:],
                    func=mybir.ActivationFunctionType.Exp,
                )
                nc.vector.tensor_tensor(
                    out=xsb[:], in0=pos[:], in1=ex[:], op=mybir.AluOpType.add
                )
                nc.vector.tensor_scalar_add(xsb[:], xsb[:], -1.0)

            for t in range(NT):
                ps = psum.tile([P, DENSE_W], f32, tag="ps")
                nc.tensor.matmul(
                    out=ps[:],
                    lhsT=xsb[:, t * P : (t + 1) * P],
                    rhs=wsb[:],
                    start=True,
                    stop=True,
                )
                st = work.tile([P, DENSE_W], f32, tag="st")
                nc.vector.tensor_copy(out=st[:], in_=ps[:])
                nc.sync.dma_start(out=OUTD[t * P : (t + 1) * P, :], in_=st[:])
    return nc


# ---------------------------------------------------------------- run layer
def _run_spmd(nc, in_maps, collect, label):
    from concourse.bass_utils import run_bass_kernel_spmd

    trace = bool(int(os.environ.get("GAT_TRACE", "0")))
    res = run_bass_kernel_spmd(
        nc, in_maps, core_ids=list(range(CORES)), trace=trace
    )
    if collect is not None:
        collect.append((label, getattr(res, "exec_time_ns", None)))
    return res.results


def _dense_phase(x, Waug, collect, label, elu_in=False):
    xT = np.ascontiguousarray(x.T.astype(np.float32))
    xT_pad = np.zeros((P, NT * P), dtype=np.float32)
    in_maps = []
    for c in range(CORES):
        xc = np.array(xT_pad)
        xc[:, :NPC] = xT[:, c * NPC : (c + 1) * NPC]
        in_maps.append({"xt": xc, "waug": Waug})
    outs = _run_spmd(build_dense_nc(elu_in), in_maps, collect, label)
    return np.concatenate([o["outd"][:NPC] for o in outs], axis=0)


def _edge_phase_v4(dense_full, plan, RWE, H, D, elu, collect, label):
    import ml_dtypes

    ZW = H * D
    ng = plan["ng"]
    ver0 = os.environ.get("GAT_V4_VER", "v6")
    tbl = np.zeros((N, RWE), dtype=ml_dtypes.bfloat16)
    tbl[:, :ZW] = dense_full[:, :ZW].astype(ml_dtypes.bfloat16)
    if ver0 != "v6":
        tbl[:, ZW : ZW + H] = dense_full[:, ZW : ZW + H].astype(
            ml_dtypes.bfloat16)
    iota = np.broadcast_to(
        np.arange(P, dtype=np.float32)[None, :], (P, P)
    ).astype(ml_dtypes.bfloat16)
    ident = np.eye(P, dtype=np.float32).astype(ml_dtypes.bfloat16)
    ver = os.environ.get("GAT_V4_VER", "v6")
    in_maps = []
    for c in range(CORES):
        pc = plan["cores"][c]
        edc = np.zeros((ng * P, H), dtype=ml_dtypes.bfloat16)
        edc[:NPC] = dense_full[
            c * NPC : (c + 1) * NPC, ZW + H : ZW + 2 * H
        ].astype(ml_dtypes.bfloat16)
        if ver == "v6":
            esl = dense_full[:, ZW : ZW + H]                  # es per node
            edl = np.zeros((ng * P, H), dtype=np.float32)
            edl[:NPC] = dense_full[
                c * NPC : (c + 1) * NPC, ZW + H : ZW + 2 * H
            ]
            td = pc["tokdst"]
            tsrc = pc["toksrc"]
            etok = np.where(
                (td >= 0)[:, :, None],
                esl[tsrc] + edl[np.maximum(td, 0)],
                -1.0e4,
            )
            etok = np.ascontiguousarray(
                etok.reshape(P, -1)).astype(ml_dtypes.bfloat16)
            in_maps.append(
                {
                    "tbl": tbl,
                    "idxa": pc["idxA"],
                    "idxb": pc["idxB"],
                    "ohs": pc["oh"],
                    "etok": etok,
                }
            )
        elif ver == "v5":
            edl = np.zeros((ng * P, H), dtype=np.float32)
            edl[:NPC] = dense_full[
                c * NPC : (c + 1) * NPC, ZW + H : ZW + 2 * H
            ]
            td = pc["tokdst"]
            edt = np.where(
                (td >= 0)[:, :, None], edl[np.maximum(td, 0)], 0.0
            )
            edt = np.ascontiguousarray(
                edt.reshape(P, -1)).astype(ml_dtypes.bfloat16)
            in_maps.append(
                {
                    "tbl": tbl,
                    "idxa": pc["idxA"],
                    "idxb": pc["idxB"],
                    "ohs": pc["oh"],
                    "edt": edt,
                }
            )
        else:
            in_maps.append(
                {
                    "tbl": tbl,
                    "idxa": pc["idxA"],
                    "idxb": pc["idxB"],
                    "dsl": pc["dsl"],
                    "edc": edc,
                    "iota": np.ascontiguousarray(iota),
                    "ident": ident,
                }
            )
    ver = os.environ.get("GAT_V4_VER", "v6")
    if ver == "v4":
        nc = build_edge_nc_v4(plan, RWE, H, D, elu)
    elif ver == "v4b":
        nc = build_edge_nc_v4b(plan, RWE, H, D, elu)
    elif ver == "v5":
        nc = build_edge_nc_v5(plan, RWE, H, D, elu)
    else:
        nc = build_edge_nc_v6(plan, RWE, H, D, elu)
    outs = _run_spmd(nc, in_maps, collect, label)
    return np.concatenate([o["out"][:NPC] for o in outs], axis=0)


# ------------------------------------------------- edge phase v7 host glue
def _edge_phase_v7(dense_full, plan, RWE, H, D, collect, label):
    import ml_dtypes

    ZW = H * D
    ng = plan["ng"]
    tbl = np.zeros((N, RWE), dtype=ml_dtypes.bfloat16)
    tbl[:, :ZW] = dense_full[:, :ZW].astype(ml_dtypes.bfloat16)
    esl = dense_full[:, ZW:ZW + H]
    in_maps = []
    for c in range(CORES):
        pc = plan["cores"][c]
        edl = np.zeros((ng * P, H), dtype=np.float32)
        edl[:NPC] = dense_full[c * NPC:(c + 1) * NPC, ZW + H:ZW + 2 * H]
        td = pc["tokdst"]
        tsrc = pc["toksrc"]
        etok = np.where((td >= 0)[:, :, None],
                        esl[tsrc] + edl[np.maximum(td, 0)], -1.0e4)
        etok = np.ascontiguousarray(
            etok.reshape(P, -1)).astype(ml_dtypes.bfloat16)
        in_maps.append({"tbl": tbl, "idx32": pc["idx32"], "ohs": pc["oh"],
                        "etok": etok})
    nc = build_edge_nc_v7(plan, RWE, H, D)
    outs = _run_spmd(nc, in_maps, collect, label)
    return np.concatenate([o["out"][:NPC] for o in outs], axis=0)


def _edge_phase_v8(dense_full, plan, RWE, H, D, collect, label):
    import ml_dtypes

    ZW = H * D
    ng = plan["ng"]
    tbl = np.zeros((N, RWE), dtype=ml_dtypes.bfloat16)
    tbl[:, :ZW] = dense_full[:, :ZW].astype(ml_dtypes.bfloat16)
    esl = dense_full[:, ZW:ZW + H]
    in_maps = []
    for c in range(CORES):
        pc = plan["cores"][c]
        edl = np.zeros((ng * P, H), dtype=np.float32)
        edl[:NPC] = dense_full[c * NPC:(c + 1) * NPC, ZW + H:ZW + 2 * H]
        td = pc["tokdst"]
        tsrc = pc["toksrc"]
        etok = np.where((td >= 0)[:, :, None],
                        esl[tsrc] + edl[np.maximum(td, 0)], -1.0e4)
        etok = np.ascontiguousarray(
            etok.reshape(P, -1)).astype(ml_dtypes.bfloat16)
        in_maps.append({"tbl": tbl, "idxa": pc["idxA"], "idxb": pc["idxB"],
                        "ohs": pc["oh"].astype(ml_dtypes.float8_e4m3),
                        "etok": etok,
                        "ident": np.eye(P, dtype=np.float32).astype(
                            ml_dtypes.bfloat16)})
    nc = build_edge_nc_v8(plan, RWE, H, D)
    outs = _run_spmd(nc, in_maps, collect, label)
    return np.concatenate([o["out"][:NPC] for o in outs], axis=0)


def _kernel_v8(h, W1a, W2a, src, dst, _collect):
    perm = np.array([[hh * HID + dd for hh in range(HEADS)]
                     for dd in range(HID)]).reshape(-1)
    W1a_p = np.array(W1a)
    W1a_p[:, :HEADS * HID] = W1a[:, perm]
    W2a_p = np.array(W2a)
    W2a_p[:HEADS * HID, :] = W2a[perm, :]

    plan = build_plan_v4(src, dst)
    d1 = _dense_phase(h, W1a_p, _collect, "dense1")
    h1 = _edge_phase_v8(d1, plan, RWE=128, H=HEADS, D=HID,
                        collect=_collect, label="edge1")
    d2 = _dense_phase(h1, W2a_p, _collect, "dense2", elu_in=True)
    out = _edge_phase_v8(d2, plan, RWE=128, H=1, D=OUT_DIM,
                         collect=_collect, label="edge2")
    return out.astype(np.float32)


def _kernel_v7(h, W1a, W2a, src, dst, _collect):
    # d-major permutation of layer-1 hidden features: new col d*H+h_ =
    # old col h_*HID+d.  Applied to W1a's z columns and W2a's rows, so
    # the on-device layouts stay consistent and the final output is
    # unpermuted.
    perm = np.array([[hh * HID + dd for hh in range(HEADS)]
                     for dd in range(HID)]).reshape(-1)
    W1a_p = np.array(W1a)
    W1a_p[:, :HEADS * HID] = W1a[:, perm]
    W2a_p = np.array(W2a)
    W2a_p[:HEADS * HID, :] = W2a[perm, :]

    plan = build_plan_v7(src, dst)
    d1 = _dense_phase(h, W1a_p, _collect, "dense1")
    h1 = _edge_phase_v7(d1, plan, RWE=128, H=HEADS, D=HID,
                        collect=_collect, label="edge1")
    d2 = _dense_phase(h1, W2a_p, _collect, "dense2", elu_in=True)
    out = _edge_phase_v7(d2, plan, RWE=128, H=1, D=OUT_DIM,
                         collect=_collect, label="edge2")
    return out.astype(np.float32)


# ---------------------------------------------------------------- kernel
def kernel(h, W1, a1_src, a1_dst, W2, a2_src, a2_dst, src, dst, _collect=None):
    h = np.asarray(h, dtype=np.float32)
    W1 = np.asarray(W1, dtype=np.float32)
    W2 = np.asarray(W2, dtype=np.float32)
    a1_src = np.asarray(a1_src, dtype=np.float32)
    a1_dst = np.asarray(a1_dst, dtype=np.float32)
    a2_src = np.asarray(a2_src, dtype=np.float32)
    a2_dst = np.asarray(a2_dst, dtype=np.float32)
    src = np.asarray(src)
    dst = np.asarray(dst)

    W1a = fuse_weights(W1, a1_src, a1_dst, HEADS, HID)
    W2a = fuse_weights(W2, a2_src, a2_dst, 1, OUT_DIM)

    ver = os.environ.get("GAT_V4_VER", "v8")
    if ver == "v8":
        return _kernel_v8(h, W1a, W2a, src, dst, _collect)
    if ver == "v7":
        return _kernel_v7(h, W1a, W2a, src, dst, _collect)
    plan = build_plan_v4(src, dst)
    rw1 = 128 if ver == "v6" else 256
    rw2 = 128
    elu1 = ver != "v6"
    d1 = _dense_phase(h, W1a, _collect, "dense1")
    h1 = _edge_phase_v4(d1, plan, RWE=rw1, H=HEADS, D=HID, elu=elu1,
                        collect=_collect, label="edge1")
    d2 = _dense_phase(h1, W2a, _collect, "dense2",
                      elu_in=(ver == "v6"))
    out = _edge_phase_v4(d2, plan, RWE=rw2, H=1, D=OUT_DIM, elu=False,
                         collect=_collect, label="edge2")
    return out.astype(np.float32)

